# revision 6
# baseline (speedup 1.0000x reference)
"""Trainium2 Bass kernel for nn_CrossAttention (B=4, C=256, H=W=48, heads=4).

Sharding: 8 cores = 4 batches x 2 halves. Queries split by row-half per
core; raw k/v split by row-half and exchanged on-device via a pair
AllGather; the folded 1x1-conv weights are sharded 1/8 per core and
8-way AllGathered. All per-core host data is packed into one bf16
tensor plus one small f32 tensor so each call stages two parameters.
The positional depthwise 3x3 conv runs on the vector engine as nine
shifted multiply-accumulates. The bf16 output is 8-way AllGathered on
device so the host fetches a single core's (full) output shard.
"""

import numpy as np
import ml_dtypes

import concourse.bass as bass
import concourse.mybir as mybir
import concourse.tile as tile
from concourse import bacc

F32 = mybir.dt.float32
BF16 = mybir.dt.bfloat16

C = 256
H = W = 48
NK = H * W            # 2304 keys
KC = NK // 128        # 18 key chunks
HEADS = 4
HD = 64
ROWS_HALF = 24        # rows per core
NQ = ROWS_HALF * W    # 1152 query positions per core
QS = 384              # query slice (8 rows)
NQS = NQ // QS        # 3 slices
QROWS = ROWS_HALF + 2  # 26 rows incl halo
NQH = QROWS * W       # 1248
NKH = NK // 2         # 1152 keys per core before exchange
EPS = 1e-5

# big (bf16) input layout, element offsets
OQ = 0
NQX = C * NQH                 # 319488
OKV = OQ + NQX
NKV = 2 * C * NKH             # 589824
OW = OKV + NKV
NW = 4 * C * C // 8           # 32768 (1/8 of the four weight matrices)
OH = OW + NW
NH = NQH
OT = OH + NH
NT = 2 * 128
XLEN = OT + NT                # 943584

# small (f32) input layout
SK = 0                        # tk  [2,128]
SV = SK + 256                 # tv  [264]  (4 heads x 66, bias in vf layout)
SP = SV + 264                 # tp  [2,128]
SJ = SP + 256                 # tj  [2,128]
SW9 = SJ + 256                # w9  [2,9,128]
SLEN = SW9 + 2304             # 3336

_CACHE = {}


def _build():
    nc = bacc.Bacc("TRN2", target_bir_lowering=False, num_devices=8)
    big = nc.dram_tensor("big", [XLEN], BF16, kind="ExternalInput")
    small = nc.dram_tensor("small", [SLEN], F32, kind="ExternalInput")
    o = nc.dram_tensor("o", [8, C, NQ], BF16, kind="ExternalOutput")

    with tile.TileContext(nc) as tc:
        with (
            tc.tile_pool(name="dram", bufs=1, space="DRAM") as dram,
            tc.tile_pool(name="wp", bufs=1) as wp,
            tc.tile_pool(name="inp", bufs=1) as inp,
            tc.tile_pool(name="feat", bufs=1) as feat,
            tc.tile_pool(name="vfp", bufs=18) as vfp,
            tc.tile_pool(name="et", bufs=40) as etp,
            tc.tile_pool(name="small", bufs=3) as smp,
            tc.tile_pool(name="ps_s", bufs=2, space="PSUM") as ps_s,
            tc.tile_pool(name="ps_w", bufs=2, space="PSUM") as ps_w,
        ):
            # ---- weight AllGather: 1/8 slice per core -> full four matrices ----
            win = dram.tile([1, NW], BF16, tag="win")
            nc.gpsimd.dma_start(
                out=win[:], in_=big[OW:OW + NW].rearrange("(x n) -> x n", x=1))
            wall = dram.tile([4, 2, 128, C], BF16, tag="wall", addr_space="Shared")
            nc.gpsimd.collective_compute(
                "AllGather", mybir.AluOpType.bypass,
                replica_groups=[[0, 1, 2, 3, 4, 5, 6, 7]],
                ins=[win.opt()], outs=[wall.opt()],
            )
            # ---- k/v pair AllGather: own key half -> both halves ----
            kvin = dram.tile([2, 2, 128, NKH], BF16, tag="kvin")
            nc.gpsimd.dma_start(
                out=kvin[:],
                in_=big[OKV:OKV + NKV].rearrange("(t a p n) -> t a p n", t=2, a=2, p=128))
            kvout = dram.tile([2, 2, 2, 128, NKH], BF16, tag="kvout")
            nc.gpsimd.collective_compute(
                "AllGather", mybir.AluOpType.bypass,
                replica_groups=[[0, 1], [2, 3], [4, 5], [6, 7]],
                ins=[kvin.opt()], outs=[kvout.opt()],
            )

            # ---- inputs / weights to SBUF ----
            q_sb = inp.tile([128, 2, NQH], BF16, tag="q")
            nc.sync.dma_start(
                out=q_sb[:], in_=big[OQ:OQ + NQX].rearrange("(a p n) -> p a n", p=128, n=NQH))
            hq_sb = wp.tile([1, NQH], BF16, tag="hq")
            nc.sync.dma_start(
                out=hq_sb[:], in_=big[OH:OH + NH].rearrange("(x n) -> x n", x=1))
            tq_sb = wp.tile([1, 2, 128], BF16, tag="tq")
            nc.sync.dma_start(
                out=tq_sb[:], in_=big[OT:OT + NT].rearrange("(x a n) -> x a n", x=1, a=2))

            w_q = wp.tile([128, 2, C], BF16, tag="wq")
            w_k = wp.tile([128, 2, C], BF16, tag="wk")
            w_v = wp.tile([128, 2, C], BF16, tag="wv")
            w_p = wp.tile([128, 2, C], BF16, tag="wpj")
            for wi, t in enumerate((w_q, w_k, w_v, w_p)):
                nc.sync.dma_start(out=t[:], in_=wall[wi].rearrange("a p n -> p a n"))

            k_sb = inp.tile([128, 2, NK], BF16, tag="k")
            v_sb = inp.tile([128, 2, NK], BF16, tag="v")
            for hh in range(2):
                nc.sync.dma_start(
                    out=k_sb[:, :, hh * NKH:(hh + 1) * NKH],
                    in_=kvout[hh, 0].rearrange("a p n -> p a n"))
                nc.sync.dma_start(
                    out=v_sb[:, :, hh * NKH:(hh + 1) * NKH],
                    in_=kvout[hh, 1].rearrange("a p n -> p a n"))

            tk_sb = wp.tile([128, 2, 1], F32, tag="tk")
            nc.sync.dma_start(
                out=tk_sb[:], in_=small[SK:SK + 256].rearrange("(a p x) -> p a x", p=128, x=1))
            tp_sb = wp.tile([128, 2, 1], F32, tag="tp")
            nc.sync.dma_start(
                out=tp_sb[:], in_=small[SP:SP + 256].rearrange("(a p x) -> p a x", p=128, x=1))
            tj_sb = wp.tile([128, 2, 1], F32, tag="tj")
            nc.sync.dma_start(
                out=tj_sb[:], in_=small[SJ:SJ + 256].rearrange("(a p x) -> p a x", p=128, x=1))
            w9_sb = wp.tile([128, 2, 9], F32, tag="w9")
            nc.sync.dma_start(
                out=w9_sb[:], in_=small[SW9:SW9 + 2304].rearrange("(a t p) -> p a t", a=2, t=9))
            tv1 = wp.tile([1, 264], F32, tag="tv1")
            nc.sync.dma_start(
                out=tv1[:], in_=small[SV:SV + 264].rearrange("(x n) -> x n", x=1))
            tv_sb = wp.tile([128, 264], F32, tag="tv")
            nc.gpsimd.partition_broadcast(tv_sb[:], tv1[:])

            # ---- qf: channel-major query features (scaled), with halo rows ----
            qf = feat.tile([128, 2, NQH], BF16, tag="qf")
            for co in range(2):
                for n0 in range(0, NQH, 512):
                    nn = min(512, NQH - n0)
                    ps = ps_w.tile([128, 512], F32, tag="w")
                    for ci in range(2):
                        nc.tensor.matmul(
                            ps[:, 0:nn],
                            w_q[:, ci, co * 128:(co + 1) * 128],
                            q_sb[:, ci, n0:n0 + nn],
                            start=(ci == 0), stop=False,
                        )
                    # masked bias: qf += tq[c] * hmask[n]  (rank-1)
                    nc.tensor.matmul(
                        ps[:, 0:nn],
                        tq_sb[:, co, :],
                        hq_sb[:, n0:n0 + nn],
                        start=False, stop=True,
                    )
                    nc.vector.tensor_copy(qf[:, co, n0:n0 + nn], ps[:, 0:nn])

            # ---- kf: channel-major key features [128, 2, NK] bf16 ----
            kf = feat.tile([128, 2, NK], BF16, tag="kf")
            for co in range(2):
                for n0 in range(0, NK, 512):
                    nn = min(512, NK - n0)
                    ps = ps_w.tile([128, 512], F32, tag="w")
                    for ci in range(2):
                        nc.tensor.matmul(
                            ps[:, 0:nn],
                            w_k[:, ci, co * 128:(co + 1) * 128],
                            k_sb[:, ci, n0:n0 + nn],
                            start=(ci == 0), stop=(ci == 1),
                        )
                    nc.vector.tensor_scalar(
                        kf[:, co, n0:n0 + nn], ps[:, 0:nn],
                        tk_sb[:, co, :], None, mybir.AluOpType.add,
                    )

            # ---- vf: position-major value features, 18 tiles [128, 4, 66] ----
            # per head h: cols [v(64) | 1 | pad]
            vf = []
            for pc in range(KC):
                vt = vfp.tile([128, 4, 66], BF16, tag="vf")
                nc.vector.memset(vt[:], 1.0)
                ps = ps_w.tile([128, 512], F32, tag="w")
                for ci in range(2):
                    nc.tensor.matmul(
                        ps[:, 0:C],
                        v_sb[:, ci, pc * 128:(pc + 1) * 128],
                        w_v[:, ci, :],
                        start=(ci == 0), stop=(ci == 1),
                    )
                psv = ps[:, 0:C].rearrange("p (h d) -> p h d", h=4)
                tvv = tv_sb[:].rearrange("p (h f) -> p h f", h=4)
                nc.vector.tensor_add(vt[:, :, 0:64], psv[:], tvv[:, :, 0:64])
                vf.append(vt)

            qfr = qf[:].rearrange("p a (r w) -> p a r w", w=W)

            # ---- attention + pe + proj, software-pipelined across q slices:
            # while ACT runs exp for slice si, PE runs AV/pe/proj of si-1.
            def emit_s_group(st, t, h):
                hp, par = h // 2, h % 2
                rs = slice(par * 64, par * 64 + 64)
                s = ps_s.tile([128, 3, 512], F32, tag="s")
                for i in range(3):
                    kc = t * 3 + i
                    nc.tensor.matmul(
                        s[:, i, 0:QS],
                        kf[rs, hp, kc * 128:(kc + 1) * 128],
                        qf[rs, hp, st["q0"]:st["q0"] + QS],
                        start=True, stop=True,
                    )
                et = etp.tile([128, 3, QS], BF16, tag="et")
                nc.scalar.activation(et[:], s[:, :, 0:QS],
                                     mybir.ActivationFunctionType.Exp)
                st["ets"][t][h] = et

            def emit_av_head(st, h):
                y = ps_w.tile([128, 512], F32, tag="w")
                for t in range(6):
                    for i in range(3):
                        kc = t * 3 + i
                        nc.tensor.matmul(
                            y[0:65, 0:QS], vf[kc][:, h, 0:65],
                            st["ets"][t][h][:, i, :],
                            start=(kc == 0), stop=(kc == KC - 1),
                        )
                st["ys"][h] = y

            def emit_norm(st, pair):
                ys = [st["ys"][pair * 2], st["ys"][pair * 2 + 1]]
                ynt = smp.tile([128, QS], BF16, tag="yn")
                rr = smp.tile([1, 2, QS], F32, tag="rr")
                rq = smp.tile([128, 2, QS], F32, tag="rq")
                for par in range(2):
                    nc.vector.reciprocal(rr[:, par, :], ys[par][64:65, 0:QS])
                nc.gpsimd.partition_broadcast(rq[:], rr[:])
                nc.vector.tensor_mul(ynt[0:64, :], ys[0][0:64, 0:QS], rq[0:64, 0, :])
                nc.vector.tensor_mul(ynt[64:128, :], ys[1][0:64, 0:QS], rq[64:128, 1, :])
                st["yn"][pair] = ynt

            # pe taps ordered so the first writes the full width (dj == 0)
            PE_TAPS = [(-1, 0), (-1, -1), (-1, 1), (0, -1), (0, 0), (0, 1),
                       (1, -1), (1, 0), (1, 1)]

            def emit_tail(st):
                r0, si = st["r0"], st["si"]
                yt = [None, None]
                for ch in range(2):
                    pet = smp.tile([128, QS], F32, tag="pe")
                    pev = pet[:].rearrange("p (r w) -> p r w", w=W)
                    for idx, (di, dj) in enumerate(PE_TAPS):
                        ti = (di + 1) * 3 + (dj + 1)
                        j0o, j0i = max(0, -dj), max(0, dj)
                        ncol = W - abs(dj)
                        src = qfr[:, ch, r0 + 1 + di:r0 + 9 + di, j0i:j0i + ncol]
                        if idx == 0:
                            nc.vector.tensor_scalar(
                                pev[:, :, j0o:j0o + ncol], src,
                                w9_sb[:, ch, ti:ti + 1], None, mybir.AluOpType.mult,
                            )
                        else:
                            nc.vector.scalar_tensor_tensor(
                                out=pev[:, :, j0o:j0o + ncol], in0=src,
                                scalar=w9_sb[:, ch, ti:ti + 1],
                                in1=pev[:, :, j0o:j0o + ncol],
                                op0=mybir.AluOpType.mult, op1=mybir.AluOpType.add,
                            )
                    ytt = smp.tile([128, QS], BF16, tag="yt")
                    nc.vector.scalar_tensor_tensor(
                        out=ytt[:], in0=pet[:], scalar=tp_sb[:, ch, :],
                        in1=st["yn"][ch][:], op0=mybir.AluOpType.add,
                        op1=mybir.AluOpType.add,
                    )
                    yt[ch] = ytt
                ob = smp.tile([128, 2, QS], BF16, tag="ob")
                for co in range(2):
                    pj = ps_w.tile([128, 512], F32, tag="w")
                    for ci in range(2):
                        nc.tensor.matmul(
                            pj[:, 0:QS],
                            w_p[:, ci, co * 128:(co + 1) * 128],
                            yt[ci][:],
                            start=(ci == 0), stop=(ci == 1),
                        )
                    nc.vector.tensor_scalar(
                        ob[:, co, :], pj[:, 0:QS], tj_sb[:, co, :], None,
                        mybir.AluOpType.add,
                    )
                nc.sync.dma_start(
                    out=opart[:].rearrange("(a p) n -> p a n", p=128)[:, :, si * QS:(si + 1) * QS],
                    in_=ob[:],
                )

            opart = dram.tile([C, NQ], BF16, tag="opart")

            FIRE = {4: lambda st: emit_av_head(st, 0),
                    8: lambda st: emit_av_head(st, 1),
                    12: lambda st: emit_norm(st, 0),
                    16: lambda st: emit_av_head(st, 2),
                    20: lambda st: emit_av_head(st, 3),
                    24: lambda st: emit_norm(st, 1)}

            prev = None
            for si in range(NQS + 1):
                cur = None
                if si < NQS:
                    cur = {"si": si, "q0": 48 + si * QS, "r0": si * (QS // W),
                           "ets": [[None] * HEADS for _ in range(6)],
                           "ys": [None] * 4, "yn": [None, None]}
                    g = 0
                    for t in range(6):
                        for h in range(HEADS):
                            emit_s_group(cur, t, h)
                            g += 1
                            if prev is not None and g in FIRE:
                                FIRE[g](prev)
                    if prev is not None:
                        emit_tail(prev)
                else:
                    for g in (4, 8, 12, 16, 20, 24):
                        FIRE[g](prev)
                    emit_tail(prev)
                prev = cur

            # ---- gather all cores' outputs so the host fetches one shard ----
            ofull = dram.tile([8, C, NQ], BF16, tag="ofull", addr_space="Shared")
            nc.gpsimd.collective_compute(
                "AllGather", mybir.AluOpType.bypass,
                replica_groups=[[0, 1, 2, 3, 4, 5, 6, 7]],
                ins=[opart.opt()], outs=[ofull.opt()],
            )
            nc.gpsimd.dma_start(out=o[:], in_=ofull[:])
    nc.compile()
    return nc


def _prep(inputs):
    """Host-side: fold BN into weights, pack per-core staged buffers."""
    f64 = np.float64
    bf = ml_dtypes.bfloat16

    def fold(w, g, b, m, v):
        s = g.astype(f64) / np.sqrt(v.astype(f64) + EPS)
        return w.astype(f64) * s[:, None], b.astype(f64) - m.astype(f64) * s

    wq, tq = fold(inputs["wq_w"], inputs["wq_g"], inputs["wq_b"], inputs["wq_m"], inputs["wq_v"])
    wk, tk = fold(inputs["wk_w"], inputs["wk_g"], inputs["wk_b"], inputs["wk_m"], inputs["wk_v"])
    wv, tv = fold(inputs["wv_w"], inputs["wv_g"], inputs["wv_b"], inputs["wv_m"], inputs["wv_v"])
    wp, tj = fold(inputs["proj_w"], inputs["proj_g"], inputs["proj_b"], inputs["proj_m"], inputs["proj_v"])
    scale = 1.0 / np.sqrt(HD)
    wq, tq = wq * scale, tq * scale
    s_pe = inputs["pe_g"].astype(f64) / np.sqrt(inputs["pe_v"].astype(f64) + EPS)
    tp = inputs["pe_b"].astype(f64) - inputs["pe_m"].astype(f64) * s_pe
    w9 = inputs["pe_w"].astype(f64).reshape(C, 9) * s_pe[:, None] / scale  # pe sees unscaled qf

    # four weight matrices, transposed, flat in [4, 2, 128, C] order
    w4 = np.empty((4, C, C), dtype=bf)
    for i, m in enumerate((wq, wk, wv, wp)):
        w4[i] = m.T.astype(bf)
    w4f = w4.reshape(4 * C * C)

    # small f32 buffer (identical on every core)
    small = np.zeros(SLEN, dtype=np.float32)
    small[SK:SK + 256] = tk.astype(np.float32)
    tvv = tv.astype(np.float32).reshape(4, 64)
    svv = small[SV:SV + 264].reshape(4, 66)
    svv[:, 0:64] = tvv
    small[SP:SP + 256] = tp.astype(np.float32)
    small[SJ:SJ + 256] = tj.astype(np.float32)
    # w9 packed (a, tap, p)
    small[SW9:SW9 + 2304] = (
        w9.reshape(2, 128, 9).transpose(0, 2, 1).astype(np.float32).reshape(-1))

    if "big" not in _CACHE:
        _CACHE["big"] = np.empty((8, XLEN), dtype=bf)
        _CACHE["small"] = np.empty((8, SLEN), dtype=np.float32)
    bigb = _CACHE["big"]
    smallb = _CACHE["small"]
    smallb[:] = small[None, :]

    q = inputs["q"].astype(bf).reshape(4, C, H, W)
    k = inputs["k"].astype(bf).reshape(4, C, H, W)
    v = inputs["v"].astype(bf).reshape(4, C, H, W)
    tqb = tq.astype(bf)

    for c in range(8):
        b, half = c // 2, c % 2
        r0 = half * ROWS_HALF
        qx = bigb[c, OQ:OQ + NQX].reshape(C, QROWS, W)
        hm = np.zeros((QROWS,), dtype=bf)
        lo, hi = max(0, r0 - 1), min(H, r0 + ROWS_HALF + 1)
        a0 = lo - (r0 - 1)
        if a0 > 0:
            qx[:, 0:a0] = 0
        if a0 + (hi - lo) < QROWS:
            qx[:, a0 + (hi - lo):] = 0
        qx[:, a0:a0 + (hi - lo)] = q[b, :, lo:hi]
        hm[a0:a0 + (hi - lo)] = 1
        kv = bigb[c, OKV:OKV + NKV].reshape(2, C, NKH)
        kv[0] = k[b, :, r0:r0 + ROWS_HALF].reshape(C, NKH)
        kv[1] = v[b, :, r0:r0 + ROWS_HALF].reshape(C, NKH)
        bigb[c, OW:OW + NW] = w4f[c * NW:(c + 1) * NW]
        bigb[c, OH:OH + NH] = np.repeat(hm, W)
        bigb[c, OT:OT + NT] = tqb
    return bigb, smallb


def _get_nc():
    if "nc" not in _CACHE:
        _CACHE["nc"] = _build()
    return _CACHE["nc"]


def _get_runner():
    if "runner" in _CACHE:
        return _CACHE["runner"]
    import jax
    from jax.sharding import Mesh, PartitionSpec
    from jax.experimental.shard_map import shard_map
    from concourse import bass2jax

    nc = _get_nc()
    bass2jax.install_neuronx_cc_hook()
    out_aval = jax.core.ShapedArray((8, C, NQ), ml_dtypes.bfloat16)

    def _body(bigv, smallv):
        outs = bass2jax._bass_exec_p.bind(
            bigv, smallv,
            out_avals=(out_aval,),
            in_names=("big", "small"),
            out_names=("o",),
            lowering_input_output_aliases=(),
            sim_require_finite=True,
            sim_require_nnan=True,
            nc=nc,
        )
        return tuple(outs)

    devices = jax.devices()[:8]
    mesh = Mesh(np.asarray(devices), ("core",))
    sharded = jax.jit(
        shard_map(
            _body, mesh=mesh,
            in_specs=(PartitionSpec("core"), PartitionSpec("core")),
            out_specs=(PartitionSpec("core"),),
            check_rep=False,
        ),
        keep_unused=True,
    )
    _CACHE["runner"] = sharded
    return sharded


def _run_fallback(big, small):
    from concourse.bass_utils import run_bass_kernel_spmd
    in_maps = [{"big": big[c], "small": small[c]} for c in range(8)]
    res = run_bass_kernel_spmd(_get_nc(), in_maps, core_ids=list(range(8)))
    return res.results[0]["o"]


def run_cores(bufs):
    big, small = bufs
    if "runner_failed" in _CACHE:
        return _run_fallback(big, small)
    try:
        sharded = _get_runner()
        out, = sharded(big.reshape(-1), small.reshape(-1))
        # every core holds the full gathered output; fetch core 0's shard only
        return np.asarray(out[0:8])
    except Exception:
        _CACHE["runner_failed"] = True
        return _run_fallback(big, small)


def assemble(out):
    # out: [8, C, NQ] bf16 (all cores' partial outputs)
    o8 = np.asarray(out).astype(np.float32).reshape(4, 2, C, ROWS_HALF, W)
    return o8.transpose(0, 2, 1, 3, 4).reshape(4, C, H, W).copy()


def kernel(**inputs):
    bufs = _prep(inputs)
    out = run_cores(bufs)
    return assemble(out)


# revision 7
# speedup vs baseline: 1.0618x; 1.0618x over previous
"""Trainium2 Bass kernel for nn_CrossAttention (B=4, C=256, H=W=48, heads=4).

Sharding: 8 cores = 4 batches x 2 halves. Queries split by row-half per
core; raw k/v split by row-half and exchanged on-device via a pair
AllGather; the folded 1x1-conv weights are sharded 1/8 per core and
8-way AllGathered. All per-core host data is packed into one bf16
tensor plus one small f32 tensor so each call stages two parameters.
The positional depthwise 3x3 conv runs on the vector engine as nine
shifted multiply-accumulates. The bf16 output is 8-way AllGathered on
device so the host fetches a single core's (full) output shard.
"""

import numpy as np
import ml_dtypes

import concourse.bass as bass
import concourse.mybir as mybir
import concourse.tile as tile
from concourse import bacc

F32 = mybir.dt.float32
BF16 = mybir.dt.bfloat16

C = 256
H = W = 48
NK = H * W            # 2304 keys
KC = NK // 128        # 18 key chunks
HEADS = 4
HD = 64
ROWS_HALF = 24        # rows per core
NQ = ROWS_HALF * W    # 1152 query positions per core
QS = 384              # query slice (8 rows)
NQS = NQ // QS        # 3 slices
QROWS = ROWS_HALF + 2  # 26 rows incl halo
NQH = QROWS * W       # 1248
NKH = NK // 2         # 1152 keys per core before exchange
EPS = 1e-5

# big (bf16) input layout, element offsets
OQ = 0
NQX = C * NQH                 # 319488
OKV = OQ + NQX
NKV = 2 * C * NKH             # 589824
OW = OKV + NKV
NW = 4 * C * C // 8           # 32768 (1/8 of the four weight matrices)
OH = OW + NW
NH = NQH
OT = OH + NH
NT = 2 * 128
XLEN = OT + NT                # 943584

# small (f32) input layout
SK = 0                        # tk  [2,128]
SV = SK + 256                 # tv  [264]  (4 heads x 66, bias in vf layout)
SP = SV + 264                 # tp  [2,128]
SJ = SP + 256                 # tj  [2,128]
SW9 = SJ + 256                # w9  [2,9,128]
SLEN = SW9 + 2304             # 3336

_CACHE = {}


def _build():
    nc = bacc.Bacc("TRN2", target_bir_lowering=False, num_devices=8)
    big = nc.dram_tensor("big", [XLEN], BF16, kind="ExternalInput")
    small = nc.dram_tensor("small", [SLEN], F32, kind="ExternalInput")
    o = nc.dram_tensor("o", [8, C, NQ], BF16, kind="ExternalOutput")

    with tile.TileContext(nc) as tc:
        with (
            tc.tile_pool(name="dram", bufs=1, space="DRAM") as dram,
            tc.tile_pool(name="wp", bufs=1) as wp,
            tc.tile_pool(name="inp", bufs=1) as inp,
            tc.tile_pool(name="feat", bufs=1) as feat,
            tc.tile_pool(name="vfp", bufs=18) as vfp,
            tc.tile_pool(name="et", bufs=40) as etp,
            tc.tile_pool(name="small", bufs=3) as smp,
            tc.tile_pool(name="ps_s", bufs=2, space="PSUM") as ps_s,
            tc.tile_pool(name="ps_w", bufs=2, space="PSUM") as ps_w,
        ):
            # ---- weight AllGather: 1/8 slice per core -> full four matrices ----
            win = dram.tile([1, NW], BF16, tag="win")
            nc.gpsimd.dma_start(
                out=win[:], in_=big[OW:OW + NW].rearrange("(x n) -> x n", x=1))
            wall = dram.tile([4, 2, 128, C], BF16, tag="wall", addr_space="Shared")
            nc.gpsimd.collective_compute(
                "AllGather", mybir.AluOpType.bypass,
                replica_groups=[[0, 1, 2, 3, 4, 5, 6, 7]],
                ins=[win.opt()], outs=[wall.opt()],
            )
            # ---- k/v pair AllGather: own key half -> both halves ----
            kvin = dram.tile([2, 2, 128, NKH], BF16, tag="kvin")
            nc.gpsimd.dma_start(
                out=kvin[:],
                in_=big[OKV:OKV + NKV].rearrange("(t a p n) -> t a p n", t=2, a=2, p=128))
            kvout = dram.tile([2, 2, 2, 128, NKH], BF16, tag="kvout")
            nc.gpsimd.collective_compute(
                "AllGather", mybir.AluOpType.bypass,
                replica_groups=[[0, 1], [2, 3], [4, 5], [6, 7]],
                ins=[kvin.opt()], outs=[kvout.opt()],
            )

            # ---- inputs / weights to SBUF ----
            q_sb = inp.tile([128, 2, NQH], BF16, tag="q")
            nc.sync.dma_start(
                out=q_sb[:], in_=big[OQ:OQ + NQX].rearrange("(a p n) -> p a n", p=128, n=NQH))
            hq_sb = wp.tile([1, NQH], BF16, tag="hq")
            nc.sync.dma_start(
                out=hq_sb[:], in_=big[OH:OH + NH].rearrange("(x n) -> x n", x=1))
            tq_sb = wp.tile([1, 2, 128], BF16, tag="tq")
            nc.sync.dma_start(
                out=tq_sb[:], in_=big[OT:OT + NT].rearrange("(x a n) -> x a n", x=1, a=2))

            w_q = wp.tile([128, 2, C], BF16, tag="wq")
            w_k = wp.tile([128, 2, C], BF16, tag="wk")
            w_v = wp.tile([128, 2, C], BF16, tag="wv")
            w_p = wp.tile([128, 2, C], BF16, tag="wpj")
            for wi, t in enumerate((w_q, w_k, w_v, w_p)):
                nc.sync.dma_start(out=t[:], in_=wall[wi].rearrange("a p n -> p a n"))

            k_sb = inp.tile([128, 2, NK], BF16, tag="k")
            v_sb = inp.tile([128, 2, NK], BF16, tag="v")
            for hh in range(2):
                nc.sync.dma_start(
                    out=k_sb[:, :, hh * NKH:(hh + 1) * NKH],
                    in_=kvout[hh, 0].rearrange("a p n -> p a n"))
                nc.sync.dma_start(
                    out=v_sb[:, :, hh * NKH:(hh + 1) * NKH],
                    in_=kvout[hh, 1].rearrange("a p n -> p a n"))

            tk_sb = wp.tile([128, 2, 1], F32, tag="tk")
            nc.sync.dma_start(
                out=tk_sb[:], in_=small[SK:SK + 256].rearrange("(a p x) -> p a x", p=128, x=1))
            tp_sb = wp.tile([128, 2, 1], F32, tag="tp")
            nc.sync.dma_start(
                out=tp_sb[:], in_=small[SP:SP + 256].rearrange("(a p x) -> p a x", p=128, x=1))
            tj_sb = wp.tile([128, 2, 1], F32, tag="tj")
            nc.sync.dma_start(
                out=tj_sb[:], in_=small[SJ:SJ + 256].rearrange("(a p x) -> p a x", p=128, x=1))
            w9_sb = wp.tile([128, 2, 9], F32, tag="w9")
            nc.sync.dma_start(
                out=w9_sb[:], in_=small[SW9:SW9 + 2304].rearrange("(a t p) -> p a t", a=2, t=9))
            tv1 = wp.tile([1, 264], F32, tag="tv1")
            nc.sync.dma_start(
                out=tv1[:], in_=small[SV:SV + 264].rearrange("(x n) -> x n", x=1))
            tv_sb = wp.tile([128, 264], F32, tag="tv")
            nc.gpsimd.partition_broadcast(tv_sb[:], tv1[:])

            # ---- qf: channel-major query features (scaled), with halo rows ----
            qf = feat.tile([128, 2, NQH], BF16, tag="qf")
            for co in range(2):
                for n0 in range(0, NQH, 512):
                    nn = min(512, NQH - n0)
                    ps = ps_w.tile([128, 512], F32, tag="w")
                    for ci in range(2):
                        nc.tensor.matmul(
                            ps[:, 0:nn],
                            w_q[:, ci, co * 128:(co + 1) * 128],
                            q_sb[:, ci, n0:n0 + nn],
                            start=(ci == 0), stop=False,
                        )
                    # masked bias: qf += tq[c] * hmask[n]  (rank-1)
                    nc.tensor.matmul(
                        ps[:, 0:nn],
                        tq_sb[:, co, :],
                        hq_sb[:, n0:n0 + nn],
                        start=False, stop=True,
                    )
                    nc.vector.tensor_copy(qf[:, co, n0:n0 + nn], ps[:, 0:nn])

            # ---- kf: channel-major key features [128, 2, NK] bf16 ----
            kf = feat.tile([128, 2, NK], BF16, tag="kf")
            for co in range(2):
                for n0 in range(0, NK, 512):
                    nn = min(512, NK - n0)
                    ps = ps_w.tile([128, 512], F32, tag="w")
                    for ci in range(2):
                        nc.tensor.matmul(
                            ps[:, 0:nn],
                            w_k[:, ci, co * 128:(co + 1) * 128],
                            k_sb[:, ci, n0:n0 + nn],
                            start=(ci == 0), stop=(ci == 1),
                        )
                    nc.vector.tensor_scalar(
                        kf[:, co, n0:n0 + nn], ps[:, 0:nn],
                        tk_sb[:, co, :], None, mybir.AluOpType.add,
                    )

            # ---- vf: position-major value features, 18 tiles [128, 4, 66] ----
            # per head h: cols [v(64) | 1 | pad]
            vf = []
            for pc in range(KC):
                vt = vfp.tile([128, 4, 66], BF16, tag="vf")
                nc.vector.memset(vt[:], 1.0)
                ps = ps_w.tile([128, 512], F32, tag="w")
                for ci in range(2):
                    nc.tensor.matmul(
                        ps[:, 0:C],
                        v_sb[:, ci, pc * 128:(pc + 1) * 128],
                        w_v[:, ci, :],
                        start=(ci == 0), stop=(ci == 1),
                    )
                psv = ps[:, 0:C].rearrange("p (h d) -> p h d", h=4)
                tvv = tv_sb[:].rearrange("p (h f) -> p h f", h=4)
                nc.vector.tensor_add(vt[:, :, 0:64], psv[:], tvv[:, :, 0:64])
                vf.append(vt)

            qfr = qf[:].rearrange("p a (r w) -> p a r w", w=W)

            # ---- attention + pe + proj, software-pipelined across q slices:
            # while ACT runs exp for slice si, PE runs AV/pe/proj of si-1.
            def emit_s_group(st, t, h):
                hp, par = h // 2, h % 2
                rs = slice(par * 64, par * 64 + 64)
                s = ps_s.tile([128, 3, 512], F32, tag="s")
                for i in range(3):
                    kc = t * 3 + i
                    nc.tensor.matmul(
                        s[:, i, 0:QS],
                        kf[rs, hp, kc * 128:(kc + 1) * 128],
                        qf[rs, hp, st["q0"]:st["q0"] + QS],
                        start=True, stop=True,
                    )
                et = etp.tile([128, 3, QS], BF16, tag="et")
                nc.scalar.activation(et[:], s[:, :, 0:QS],
                                     mybir.ActivationFunctionType.Exp)
                st["ets"][t][h] = et

            def emit_av_head(st, h):
                y = ps_w.tile([128, 512], F32, tag="w")
                for t in range(6):
                    for i in range(3):
                        kc = t * 3 + i
                        nc.tensor.matmul(
                            y[0:65, 0:QS], vf[kc][:, h, 0:65],
                            st["ets"][t][h][:, i, :],
                            start=(kc == 0), stop=(kc == KC - 1),
                        )
                st["ys"][h] = y

            def emit_norm(st, pair):
                ys = [st["ys"][pair * 2], st["ys"][pair * 2 + 1]]
                ynt = smp.tile([128, QS], BF16, tag="yn")
                rr = smp.tile([1, 2, QS], F32, tag="rr")
                rq = smp.tile([128, 2, QS], F32, tag="rq")
                for par in range(2):
                    nc.vector.reciprocal(rr[:, par, :], ys[par][64:65, 0:QS])
                nc.gpsimd.partition_broadcast(rq[:], rr[:])
                nc.vector.tensor_mul(ynt[0:64, :], ys[0][0:64, 0:QS], rq[0:64, 0, :])
                nc.vector.tensor_mul(ynt[64:128, :], ys[1][0:64, 0:QS], rq[64:128, 1, :])
                st["yn"][pair] = ynt

            # pe taps ordered so the first writes the full width (dj == 0)
            PE_TAPS = [(-1, 0), (-1, -1), (-1, 1), (0, -1), (0, 0), (0, 1),
                       (1, -1), (1, 0), (1, 1)]

            def emit_tail(st):
                r0, si = st["r0"], st["si"]
                yt = [None, None]
                for ch in range(2):
                    pet = smp.tile([128, QS], F32, tag="pe")
                    pev = pet[:].rearrange("p (r w) -> p r w", w=W)
                    for idx, (di, dj) in enumerate(PE_TAPS):
                        ti = (di + 1) * 3 + (dj + 1)
                        j0o, j0i = max(0, -dj), max(0, dj)
                        ncol = W - abs(dj)
                        src = qfr[:, ch, r0 + 1 + di:r0 + 9 + di, j0i:j0i + ncol]
                        if idx == 0:
                            nc.vector.tensor_scalar(
                                pev[:, :, j0o:j0o + ncol], src,
                                w9_sb[:, ch, ti:ti + 1], None, mybir.AluOpType.mult,
                            )
                        else:
                            nc.vector.scalar_tensor_tensor(
                                out=pev[:, :, j0o:j0o + ncol], in0=src,
                                scalar=w9_sb[:, ch, ti:ti + 1],
                                in1=pev[:, :, j0o:j0o + ncol],
                                op0=mybir.AluOpType.mult, op1=mybir.AluOpType.add,
                            )
                    ytt = smp.tile([128, QS], BF16, tag="yt")
                    nc.vector.scalar_tensor_tensor(
                        out=ytt[:], in0=pet[:], scalar=tp_sb[:, ch, :],
                        in1=st["yn"][ch][:], op0=mybir.AluOpType.add,
                        op1=mybir.AluOpType.add,
                    )
                    yt[ch] = ytt
                ob = smp.tile([128, 2, QS], BF16, tag="ob")
                for co in range(2):
                    pj = ps_w.tile([128, 512], F32, tag="w")
                    for ci in range(2):
                        nc.tensor.matmul(
                            pj[:, 0:QS],
                            w_p[:, ci, co * 128:(co + 1) * 128],
                            yt[ci][:],
                            start=(ci == 0), stop=(ci == 1),
                        )
                    nc.vector.tensor_scalar(
                        ob[:, co, :], pj[:, 0:QS], tj_sb[:, co, :], None,
                        mybir.AluOpType.add,
                    )
                nc.sync.dma_start(
                    out=opart[:].rearrange("(a p) n -> p a n", p=128)[:, :, si * QS:(si + 1) * QS],
                    in_=ob[:],
                )

            opart = dram.tile([C, NQ], BF16, tag="opart")

            FIRE = {4: lambda st: emit_av_head(st, 0),
                    8: lambda st: emit_av_head(st, 1),
                    12: lambda st: emit_norm(st, 0),
                    16: lambda st: emit_av_head(st, 2),
                    20: lambda st: emit_av_head(st, 3),
                    24: lambda st: emit_norm(st, 1)}

            prev = None
            for si in range(NQS + 1):
                cur = None
                if si < NQS:
                    cur = {"si": si, "q0": 48 + si * QS, "r0": si * (QS // W),
                           "ets": [[None] * HEADS for _ in range(6)],
                           "ys": [None] * 4, "yn": [None, None]}
                    g = 0
                    for t in range(6):
                        for h in range(HEADS):
                            emit_s_group(cur, t, h)
                            g += 1
                            if prev is not None and g in FIRE:
                                FIRE[g](prev)
                    if prev is not None:
                        emit_tail(prev)
                else:
                    for g in (4, 8, 12, 16, 20, 24):
                        FIRE[g](prev)
                    emit_tail(prev)
                prev = cur

            # ---- gather all cores' outputs so the host fetches one shard ----
            ofull = dram.tile([8, C, NQ], BF16, tag="ofull", addr_space="Shared")
            nc.gpsimd.collective_compute(
                "AllGather", mybir.AluOpType.bypass,
                replica_groups=[[0, 1, 2, 3, 4, 5, 6, 7]],
                ins=[opart.opt()], outs=[ofull.opt()],
            )
            nc.gpsimd.dma_start(out=o[:], in_=ofull[:])
    nc.compile()
    return nc


def _prep(inputs):
    """Host-side: fold BN into weights, pack per-core staged buffers."""
    f64 = np.float64
    bf = ml_dtypes.bfloat16

    def fold(w, g, b, m, v):
        s = g.astype(f64) / np.sqrt(v.astype(f64) + EPS)
        return w.astype(f64) * s[:, None], b.astype(f64) - m.astype(f64) * s

    wq, tq = fold(inputs["wq_w"], inputs["wq_g"], inputs["wq_b"], inputs["wq_m"], inputs["wq_v"])
    wk, tk = fold(inputs["wk_w"], inputs["wk_g"], inputs["wk_b"], inputs["wk_m"], inputs["wk_v"])
    wv, tv = fold(inputs["wv_w"], inputs["wv_g"], inputs["wv_b"], inputs["wv_m"], inputs["wv_v"])
    wp, tj = fold(inputs["proj_w"], inputs["proj_g"], inputs["proj_b"], inputs["proj_m"], inputs["proj_v"])
    scale = 1.0 / np.sqrt(HD)
    wq, tq = wq * scale, tq * scale
    s_pe = inputs["pe_g"].astype(f64) / np.sqrt(inputs["pe_v"].astype(f64) + EPS)
    tp = inputs["pe_b"].astype(f64) - inputs["pe_m"].astype(f64) * s_pe
    w9 = inputs["pe_w"].astype(f64).reshape(C, 9) * s_pe[:, None] / scale  # pe sees unscaled qf

    # four weight matrices, transposed, flat in [4, 2, 128, C] order
    w4 = np.empty((4, C, C), dtype=bf)
    for i, m in enumerate((wq, wk, wv, wp)):
        w4[i] = m.T.astype(bf)
    w4f = w4.reshape(4 * C * C)

    # small f32 buffer (identical on every core)
    small = np.zeros(SLEN, dtype=np.float32)
    small[SK:SK + 256] = tk.astype(np.float32)
    tvv = tv.astype(np.float32).reshape(4, 64)
    svv = small[SV:SV + 264].reshape(4, 66)
    svv[:, 0:64] = tvv
    small[SP:SP + 256] = tp.astype(np.float32)
    small[SJ:SJ + 256] = tj.astype(np.float32)
    # w9 packed (a, tap, p)
    small[SW9:SW9 + 2304] = (
        w9.reshape(2, 128, 9).transpose(0, 2, 1).astype(np.float32).reshape(-1))

    if "big" not in _CACHE:
        _CACHE["big"] = np.empty((8, XLEN), dtype=bf)
        _CACHE["small"] = np.empty((8, SLEN), dtype=np.float32)
    bigb = _CACHE["big"]
    smallb = _CACHE["small"]
    smallb[:] = small[None, :]

    q = inputs["q"].astype(bf).reshape(4, C, H, W)
    k = inputs["k"].astype(bf).reshape(4, C, H, W)
    v = inputs["v"].astype(bf).reshape(4, C, H, W)
    tqb = tq.astype(bf)

    for c in range(8):
        b, half = c // 2, c % 2
        r0 = half * ROWS_HALF
        qx = bigb[c, OQ:OQ + NQX].reshape(C, QROWS, W)
        hm = np.zeros((QROWS,), dtype=bf)
        lo, hi = max(0, r0 - 1), min(H, r0 + ROWS_HALF + 1)
        a0 = lo - (r0 - 1)
        if a0 > 0:
            qx[:, 0:a0] = 0
        if a0 + (hi - lo) < QROWS:
            qx[:, a0 + (hi - lo):] = 0
        qx[:, a0:a0 + (hi - lo)] = q[b, :, lo:hi]
        hm[a0:a0 + (hi - lo)] = 1
        kv = bigb[c, OKV:OKV + NKV].reshape(2, C, NKH)
        kv[0] = k[b, :, r0:r0 + ROWS_HALF].reshape(C, NKH)
        kv[1] = v[b, :, r0:r0 + ROWS_HALF].reshape(C, NKH)
        bigb[c, OW:OW + NW] = w4f[c * NW:(c + 1) * NW]
        bigb[c, OH:OH + NH] = np.repeat(hm, W)
        bigb[c, OT:OT + NT] = tqb
    return bigb, smallb


def _get_nc():
    if "nc" not in _CACHE:
        _CACHE["nc"] = _build()
    return _CACHE["nc"]


def _get_runner():
    if "runner" in _CACHE:
        return _CACHE["runner"]
    import jax
    from jax.sharding import Mesh, PartitionSpec
    from jax.experimental.shard_map import shard_map
    from concourse import bass2jax

    nc = _get_nc()
    bass2jax.install_neuronx_cc_hook()
    out_aval = jax.core.ShapedArray((8, C, NQ), ml_dtypes.bfloat16)

    def _body(bigv, smallv):
        outs = bass2jax._bass_exec_p.bind(
            bigv, smallv,
            out_avals=(out_aval,),
            in_names=("big", "small"),
            out_names=("o",),
            lowering_input_output_aliases=(),
            sim_require_finite=True,
            sim_require_nnan=True,
            nc=nc,
        )
        return tuple(outs)

    devices = jax.devices()[:8]
    mesh = Mesh(np.asarray(devices), ("core",))
    sharded = jax.jit(
        shard_map(
            _body, mesh=mesh,
            in_specs=(PartitionSpec("core"), PartitionSpec("core")),
            out_specs=(PartitionSpec("core"),),
            check_rep=False,
        ),
        keep_unused=True,
    )
    _CACHE["runner"] = sharded
    return sharded


def _run_fallback(big, small):
    from concourse.bass_utils import run_bass_kernel_spmd
    in_maps = [{"big": big[c], "small": small[c]} for c in range(8)]
    res = run_bass_kernel_spmd(_get_nc(), in_maps, core_ids=list(range(8)))
    return res.results[0]["o"]


def run_cores(bufs):
    big, small = bufs
    if "runner_failed" in _CACHE:
        return _run_fallback(big, small)
    try:
        sharded = _get_runner()
        out, = sharded(big.reshape(-1), small.reshape(-1))
        # every core holds the full gathered output; fetch core 0's shard only
        return np.asarray(out.addressable_data(0))
    except Exception:
        _CACHE["runner_failed"] = True
        return _run_fallback(big, small)


def assemble(out):
    # out: [8, C, NQ] bf16 (all cores' partial outputs)
    o8 = np.asarray(out).astype(np.float32).reshape(4, 2, C, ROWS_HALF, W)
    return o8.transpose(0, 2, 1, 3, 4).reshape(4, C, H, W).copy()


def kernel(**inputs):
    bufs = _prep(inputs)
    out = run_cores(bufs)
    return assemble(out)


# revision 9
# speedup vs baseline: 4.2921x; 4.0423x over previous
"""Trainium2 Bass kernel for nn_CrossAttention (B=4, C=256, H=W=48, heads=4).

Sharding: 8 cores = 4 batches x 2 halves. Queries split by row-half per
core; raw k/v split by row-half and exchanged on-device via a pair
AllGather; the folded 1x1-conv weights are sharded 1/8 per core and
8-way AllGathered. All per-core host data is packed into one bf16
tensor plus one small f32 tensor so each call stages two parameters.
The positional depthwise 3x3 conv runs on the vector engine as nine
shifted multiply-accumulates. The bf16 output is 8-way AllGathered on
device so the host fetches a single core's (full) output shard.
"""

import numpy as np
import ml_dtypes

import concourse.bass as bass
import concourse.mybir as mybir
import concourse.tile as tile
from concourse import bacc

F32 = mybir.dt.float32
BF16 = mybir.dt.bfloat16

C = 256
H = W = 48
NK = H * W            # 2304 keys
KC = NK // 128        # 18 key chunks
HEADS = 4
HD = 64
ROWS_HALF = 24        # rows per core
NQ = ROWS_HALF * W    # 1152 query positions per core
QS = 384              # query slice (8 rows)
NQS = NQ // QS        # 3 slices
QROWS = ROWS_HALF + 2  # 26 rows incl halo
NQH = QROWS * W       # 1248
NKH = NK // 2         # 1152 keys per core before exchange
EPS = 1e-5

# big (bf16) input layout, element offsets
OQ = 0
NQX = C * NQH                 # 319488
OKV = OQ + NQX
NKV = 2 * C * NKH             # 589824
OW = OKV + NKV
NW = 4 * C * C // 8           # 32768 (1/8 of the four weight matrices)
OH = OW + NW
NH = NQH
OT = OH + NH
NT = 2 * 128
XLEN = OT + NT                # 943584

# small (f32) input layout
SK = 0                        # tk  [2,128]
SV = SK + 256                 # tv  [264]  (4 heads x 66, bias in vf layout)
SP = SV + 264                 # tp  [2,128]
SJ = SP + 256                 # tj  [2,128]
SW9 = SJ + 256                # w9  [2,9,128]
SLEN = SW9 + 2304             # 3336

_CACHE = {}


def _build():
    nc = bacc.Bacc("TRN2", target_bir_lowering=False, num_devices=8)
    big = nc.dram_tensor("big", [XLEN], BF16, kind="ExternalInput")
    small = nc.dram_tensor("small", [SLEN], F32, kind="ExternalInput")
    o = nc.dram_tensor("o", [8, C, NQ], BF16, kind="ExternalOutput")

    with tile.TileContext(nc) as tc:
        with (
            tc.tile_pool(name="dram", bufs=1, space="DRAM") as dram,
            tc.tile_pool(name="wp", bufs=1) as wp,
            tc.tile_pool(name="inp", bufs=1) as inp,
            tc.tile_pool(name="feat", bufs=1) as feat,
            tc.tile_pool(name="vfp", bufs=18) as vfp,
            tc.tile_pool(name="et", bufs=40) as etp,
            tc.tile_pool(name="small", bufs=3) as smp,
            tc.tile_pool(name="ps_s", bufs=2, space="PSUM") as ps_s,
            tc.tile_pool(name="ps_w", bufs=2, space="PSUM") as ps_w,
        ):
            # ---- weight AllGather: 1/8 slice per core -> full four matrices ----
            win = dram.tile([1, NW], BF16, tag="win")
            nc.gpsimd.dma_start(
                out=win[:], in_=big[OW:OW + NW].rearrange("(x n) -> x n", x=1))
            wall = dram.tile([4, 2, 128, C], BF16, tag="wall", addr_space="Shared")
            nc.gpsimd.collective_compute(
                "AllGather", mybir.AluOpType.bypass,
                replica_groups=[[0, 1, 2, 3, 4, 5, 6, 7]],
                ins=[win.opt()], outs=[wall.opt()],
            )
            # ---- k/v pair AllGather: own key half -> both halves ----
            kvin = dram.tile([2, 2, 128, NKH], BF16, tag="kvin")
            nc.gpsimd.dma_start(
                out=kvin[:],
                in_=big[OKV:OKV + NKV].rearrange("(t a p n) -> t a p n", t=2, a=2, p=128))
            kvout = dram.tile([2, 2, 2, 128, NKH], BF16, tag="kvout")
            nc.gpsimd.collective_compute(
                "AllGather", mybir.AluOpType.bypass,
                replica_groups=[[0, 1], [2, 3], [4, 5], [6, 7]],
                ins=[kvin.opt()], outs=[kvout.opt()],
            )

            # ---- inputs / weights to SBUF ----
            q_sb = inp.tile([128, 2, NQH], BF16, tag="q")
            nc.sync.dma_start(
                out=q_sb[:], in_=big[OQ:OQ + NQX].rearrange("(a p n) -> p a n", p=128, n=NQH))
            hq_sb = wp.tile([1, NQH], BF16, tag="hq")
            nc.sync.dma_start(
                out=hq_sb[:], in_=big[OH:OH + NH].rearrange("(x n) -> x n", x=1))
            tq_sb = wp.tile([1, 2, 128], BF16, tag="tq")
            nc.sync.dma_start(
                out=tq_sb[:], in_=big[OT:OT + NT].rearrange("(x a n) -> x a n", x=1, a=2))

            w_q = wp.tile([128, 2, C], BF16, tag="wq")
            w_k = wp.tile([128, 2, C], BF16, tag="wk")
            w_v = wp.tile([128, 2, C], BF16, tag="wv")
            w_p = wp.tile([128, 2, C], BF16, tag="wpj")
            for wi, t in enumerate((w_q, w_k, w_v, w_p)):
                nc.sync.dma_start(out=t[:], in_=wall[wi].rearrange("a p n -> p a n"))

            k_sb = inp.tile([128, 2, NK], BF16, tag="k")
            v_sb = inp.tile([128, 2, NK], BF16, tag="v")
            for hh in range(2):
                nc.sync.dma_start(
                    out=k_sb[:, :, hh * NKH:(hh + 1) * NKH],
                    in_=kvout[hh, 0].rearrange("a p n -> p a n"))
                nc.sync.dma_start(
                    out=v_sb[:, :, hh * NKH:(hh + 1) * NKH],
                    in_=kvout[hh, 1].rearrange("a p n -> p a n"))

            tk_sb = wp.tile([128, 2, 1], F32, tag="tk")
            nc.sync.dma_start(
                out=tk_sb[:], in_=small[SK:SK + 256].rearrange("(a p x) -> p a x", p=128, x=1))
            tp_sb = wp.tile([128, 2, 1], F32, tag="tp")
            nc.sync.dma_start(
                out=tp_sb[:], in_=small[SP:SP + 256].rearrange("(a p x) -> p a x", p=128, x=1))
            tj_sb = wp.tile([128, 2, 1], F32, tag="tj")
            nc.sync.dma_start(
                out=tj_sb[:], in_=small[SJ:SJ + 256].rearrange("(a p x) -> p a x", p=128, x=1))
            w9_sb = wp.tile([128, 2, 9], F32, tag="w9")
            nc.sync.dma_start(
                out=w9_sb[:], in_=small[SW9:SW9 + 2304].rearrange("(a t p) -> p a t", a=2, t=9))
            tv1 = wp.tile([1, 264], F32, tag="tv1")
            nc.sync.dma_start(
                out=tv1[:], in_=small[SV:SV + 264].rearrange("(x n) -> x n", x=1))
            tv_sb = wp.tile([128, 264], F32, tag="tv")
            nc.gpsimd.partition_broadcast(tv_sb[:], tv1[:])

            # ---- qf: channel-major query features (scaled), with halo rows ----
            qf = feat.tile([128, 2, NQH], BF16, tag="qf")
            for co in range(2):
                for n0 in range(0, NQH, 512):
                    nn = min(512, NQH - n0)
                    ps = ps_w.tile([128, 512], F32, tag="w")
                    for ci in range(2):
                        nc.tensor.matmul(
                            ps[:, 0:nn],
                            w_q[:, ci, co * 128:(co + 1) * 128],
                            q_sb[:, ci, n0:n0 + nn],
                            start=(ci == 0), stop=False,
                        )
                    # masked bias: qf += tq[c] * hmask[n]  (rank-1)
                    nc.tensor.matmul(
                        ps[:, 0:nn],
                        tq_sb[:, co, :],
                        hq_sb[:, n0:n0 + nn],
                        start=False, stop=True,
                    )
                    nc.vector.tensor_copy(qf[:, co, n0:n0 + nn], ps[:, 0:nn])

            # ---- kf: channel-major key features [128, 2, NK] bf16 ----
            kf = feat.tile([128, 2, NK], BF16, tag="kf")
            for co in range(2):
                for n0 in range(0, NK, 512):
                    nn = min(512, NK - n0)
                    ps = ps_w.tile([128, 512], F32, tag="w")
                    for ci in range(2):
                        nc.tensor.matmul(
                            ps[:, 0:nn],
                            w_k[:, ci, co * 128:(co + 1) * 128],
                            k_sb[:, ci, n0:n0 + nn],
                            start=(ci == 0), stop=(ci == 1),
                        )
                    nc.vector.tensor_scalar(
                        kf[:, co, n0:n0 + nn], ps[:, 0:nn],
                        tk_sb[:, co, :], None, mybir.AluOpType.add,
                    )

            # ---- vf: position-major value features, 18 tiles [128, 4, 66] ----
            # per head h: cols [v(64) | 1 | pad]
            vf = []
            for pc in range(KC):
                vt = vfp.tile([128, 4, 66], BF16, tag="vf")
                nc.vector.memset(vt[:], 1.0)
                ps = ps_w.tile([128, 512], F32, tag="w")
                for ci in range(2):
                    nc.tensor.matmul(
                        ps[:, 0:C],
                        v_sb[:, ci, pc * 128:(pc + 1) * 128],
                        w_v[:, ci, :],
                        start=(ci == 0), stop=(ci == 1),
                    )
                psv = ps[:, 0:C].rearrange("p (h d) -> p h d", h=4)
                tvv = tv_sb[:].rearrange("p (h f) -> p h f", h=4)
                nc.vector.tensor_add(vt[:, :, 0:64], psv[:], tvv[:, :, 0:64])
                vf.append(vt)

            qfr = qf[:].rearrange("p a (r w) -> p a r w", w=W)

            # ---- attention + pe + proj, software-pipelined across q slices:
            # while ACT runs exp for slice si, PE runs AV/pe/proj of si-1.
            def emit_s_group(st, t, h):
                hp, par = h // 2, h % 2
                rs = slice(par * 64, par * 64 + 64)
                s = ps_s.tile([128, 3, 512], F32, tag="s")
                for i in range(3):
                    kc = t * 3 + i
                    nc.tensor.matmul(
                        s[:, i, 0:QS],
                        kf[rs, hp, kc * 128:(kc + 1) * 128],
                        qf[rs, hp, st["q0"]:st["q0"] + QS],
                        start=True, stop=True,
                    )
                et = etp.tile([128, 3, QS], BF16, tag="et")
                nc.scalar.activation(et[:], s[:, :, 0:QS],
                                     mybir.ActivationFunctionType.Exp)
                st["ets"][t][h] = et

            def emit_av_head(st, h):
                y = ps_w.tile([128, 512], F32, tag="w")
                for t in range(6):
                    for i in range(3):
                        kc = t * 3 + i
                        nc.tensor.matmul(
                            y[0:65, 0:QS], vf[kc][:, h, 0:65],
                            st["ets"][t][h][:, i, :],
                            start=(kc == 0), stop=(kc == KC - 1),
                        )
                st["ys"][h] = y

            def emit_norm(st, pair):
                ys = [st["ys"][pair * 2], st["ys"][pair * 2 + 1]]
                ynt = smp.tile([128, QS], BF16, tag="yn")
                rr = smp.tile([1, 2, QS], F32, tag="rr")
                rq = smp.tile([128, 2, QS], F32, tag="rq")
                for par in range(2):
                    nc.vector.reciprocal(rr[:, par, :], ys[par][64:65, 0:QS])
                nc.gpsimd.partition_broadcast(rq[:], rr[:])
                nc.vector.tensor_mul(ynt[0:64, :], ys[0][0:64, 0:QS], rq[0:64, 0, :])
                nc.vector.tensor_mul(ynt[64:128, :], ys[1][0:64, 0:QS], rq[64:128, 1, :])
                st["yn"][pair] = ynt

            # pe taps ordered so the first writes the full width (dj == 0)
            PE_TAPS = [(-1, 0), (-1, -1), (-1, 1), (0, -1), (0, 0), (0, 1),
                       (1, -1), (1, 0), (1, 1)]

            def emit_tail(st):
                r0, si = st["r0"], st["si"]
                yt = [None, None]
                for ch in range(2):
                    pet = smp.tile([128, QS], F32, tag="pe")
                    pev = pet[:].rearrange("p (r w) -> p r w", w=W)
                    for idx, (di, dj) in enumerate(PE_TAPS):
                        ti = (di + 1) * 3 + (dj + 1)
                        j0o, j0i = max(0, -dj), max(0, dj)
                        ncol = W - abs(dj)
                        src = qfr[:, ch, r0 + 1 + di:r0 + 9 + di, j0i:j0i + ncol]
                        if idx == 0:
                            nc.vector.tensor_scalar(
                                pev[:, :, j0o:j0o + ncol], src,
                                w9_sb[:, ch, ti:ti + 1], None, mybir.AluOpType.mult,
                            )
                        else:
                            nc.vector.scalar_tensor_tensor(
                                out=pev[:, :, j0o:j0o + ncol], in0=src,
                                scalar=w9_sb[:, ch, ti:ti + 1],
                                in1=pev[:, :, j0o:j0o + ncol],
                                op0=mybir.AluOpType.mult, op1=mybir.AluOpType.add,
                            )
                    ytt = smp.tile([128, QS], BF16, tag="yt")
                    nc.vector.scalar_tensor_tensor(
                        out=ytt[:], in0=pet[:], scalar=tp_sb[:, ch, :],
                        in1=st["yn"][ch][:], op0=mybir.AluOpType.add,
                        op1=mybir.AluOpType.add,
                    )
                    yt[ch] = ytt
                ob = smp.tile([128, 2, QS], BF16, tag="ob")
                for co in range(2):
                    pj = ps_w.tile([128, 512], F32, tag="w")
                    for ci in range(2):
                        nc.tensor.matmul(
                            pj[:, 0:QS],
                            w_p[:, ci, co * 128:(co + 1) * 128],
                            yt[ci][:],
                            start=(ci == 0), stop=(ci == 1),
                        )
                    nc.vector.tensor_scalar(
                        ob[:, co, :], pj[:, 0:QS], tj_sb[:, co, :], None,
                        mybir.AluOpType.add,
                    )
                nc.sync.dma_start(
                    out=opart[:].rearrange("(a p) n -> p a n", p=128)[:, :, si * QS:(si + 1) * QS],
                    in_=ob[:],
                )

            opart = dram.tile([C, NQ], BF16, tag="opart")

            FIRE = {4: lambda st: emit_av_head(st, 0),
                    8: lambda st: emit_av_head(st, 1),
                    12: lambda st: emit_norm(st, 0),
                    16: lambda st: emit_av_head(st, 2),
                    20: lambda st: emit_av_head(st, 3),
                    24: lambda st: emit_norm(st, 1)}

            prev = None
            for si in range(NQS + 1):
                cur = None
                if si < NQS:
                    cur = {"si": si, "q0": 48 + si * QS, "r0": si * (QS // W),
                           "ets": [[None] * HEADS for _ in range(6)],
                           "ys": [None] * 4, "yn": [None, None]}
                    g = 0
                    for t in range(6):
                        for h in range(HEADS):
                            emit_s_group(cur, t, h)
                            g += 1
                            if prev is not None and g in FIRE:
                                FIRE[g](prev)
                    if prev is not None:
                        emit_tail(prev)
                else:
                    for g in (4, 8, 12, 16, 20, 24):
                        FIRE[g](prev)
                    emit_tail(prev)
                prev = cur

            # ---- gather all cores' outputs so the host fetches one shard ----
            ofull = dram.tile([8, C, NQ], BF16, tag="ofull", addr_space="Shared")
            nc.gpsimd.collective_compute(
                "AllGather", mybir.AluOpType.bypass,
                replica_groups=[[0, 1, 2, 3, 4, 5, 6, 7]],
                ins=[opart.opt()], outs=[ofull.opt()],
            )
            nc.gpsimd.dma_start(out=o[:], in_=ofull[:])
    nc.compile()
    return nc


def _prep(inputs):
    """Host-side: fold BN into weights, pack per-core staged buffers."""
    f64 = np.float64
    bf = ml_dtypes.bfloat16

    def fold(w, g, b, m, v):
        s = g.astype(f64) / np.sqrt(v.astype(f64) + EPS)
        return w.astype(f64) * s[:, None], b.astype(f64) - m.astype(f64) * s

    wq, tq = fold(inputs["wq_w"], inputs["wq_g"], inputs["wq_b"], inputs["wq_m"], inputs["wq_v"])
    wk, tk = fold(inputs["wk_w"], inputs["wk_g"], inputs["wk_b"], inputs["wk_m"], inputs["wk_v"])
    wv, tv = fold(inputs["wv_w"], inputs["wv_g"], inputs["wv_b"], inputs["wv_m"], inputs["wv_v"])
    wp, tj = fold(inputs["proj_w"], inputs["proj_g"], inputs["proj_b"], inputs["proj_m"], inputs["proj_v"])
    scale = 1.0 / np.sqrt(HD)
    wq, tq = wq * scale, tq * scale
    s_pe = inputs["pe_g"].astype(f64) / np.sqrt(inputs["pe_v"].astype(f64) + EPS)
    tp = inputs["pe_b"].astype(f64) - inputs["pe_m"].astype(f64) * s_pe
    w9 = inputs["pe_w"].astype(f64).reshape(C, 9) * s_pe[:, None] / scale  # pe sees unscaled qf

    # four weight matrices, transposed, flat in [4, 2, 128, C] order
    w4 = np.empty((4, C, C), dtype=bf)
    for i, m in enumerate((wq, wk, wv, wp)):
        w4[i] = m.T.astype(bf)
    w4f = w4.reshape(4 * C * C)

    # small f32 buffer (identical on every core)
    small = np.zeros(SLEN, dtype=np.float32)
    small[SK:SK + 256] = tk.astype(np.float32)
    tvv = tv.astype(np.float32).reshape(4, 64)
    svv = small[SV:SV + 264].reshape(4, 66)
    svv[:, 0:64] = tvv
    small[SP:SP + 256] = tp.astype(np.float32)
    small[SJ:SJ + 256] = tj.astype(np.float32)
    # w9 packed (a, tap, p)
    small[SW9:SW9 + 2304] = (
        w9.reshape(2, 128, 9).transpose(0, 2, 1).astype(np.float32).reshape(-1))

    if "big" not in _CACHE:
        _CACHE["big"] = np.empty((8, XLEN), dtype=bf)
        _CACHE["small"] = np.empty((8, SLEN), dtype=np.float32)
    bigb = _CACHE["big"]
    smallb = _CACHE["small"]
    smallb[:] = small[None, :]

    q = inputs["q"].astype(bf).reshape(4, C, H, W)
    k = inputs["k"].astype(bf).reshape(4, C, H, W)
    v = inputs["v"].astype(bf).reshape(4, C, H, W)
    tqb = tq.astype(bf)

    for c in range(8):
        b, half = c // 2, c % 2
        r0 = half * ROWS_HALF
        qx = bigb[c, OQ:OQ + NQX].reshape(C, QROWS, W)
        hm = np.zeros((QROWS,), dtype=bf)
        lo, hi = max(0, r0 - 1), min(H, r0 + ROWS_HALF + 1)
        a0 = lo - (r0 - 1)
        if a0 > 0:
            qx[:, 0:a0] = 0
        if a0 + (hi - lo) < QROWS:
            qx[:, a0 + (hi - lo):] = 0
        qx[:, a0:a0 + (hi - lo)] = q[b, :, lo:hi]
        hm[a0:a0 + (hi - lo)] = 1
        kv = bigb[c, OKV:OKV + NKV].reshape(2, C, NKH)
        kv[0] = k[b, :, r0:r0 + ROWS_HALF].reshape(C, NKH)
        kv[1] = v[b, :, r0:r0 + ROWS_HALF].reshape(C, NKH)
        bigb[c, OW:OW + NW] = w4f[c * NW:(c + 1) * NW]
        bigb[c, OH:OH + NH] = np.repeat(hm, W)
        bigb[c, OT:OT + NT] = tqb
    return bigb, smallb


def _get_nc():
    if "nc" not in _CACHE:
        _CACHE["nc"] = _build()
    return _CACHE["nc"]


def _get_runner():
    if "runner" in _CACHE:
        return _CACHE["runner"]
    import jax
    from jax.sharding import Mesh, PartitionSpec
    from jax.experimental.shard_map import shard_map
    from concourse import bass2jax

    nc = _get_nc()
    bass2jax.install_neuronx_cc_hook()
    out_aval = jax.core.ShapedArray((8, C, NQ), ml_dtypes.bfloat16)
    pid_name = nc.partition_id_tensor.name if nc.partition_id_tensor else None
    in_names = ("big", "small") + ((pid_name,) if pid_name else ())

    def _body(bigv, smallv):
        operands = [bigv, smallv]
        if pid_name is not None:
            operands.append(bass2jax.partition_id_tensor())
        outs = bass2jax._bass_exec_p.bind(
            *operands,
            out_avals=(out_aval,),
            in_names=in_names,
            out_names=("o",),
            lowering_input_output_aliases=(),
            sim_require_finite=True,
            sim_require_nnan=True,
            nc=nc,
        )
        return tuple(outs)

    devices = jax.devices()[:8]
    mesh = Mesh(np.asarray(devices), ("core",))
    sharded = jax.jit(
        shard_map(
            _body, mesh=mesh,
            in_specs=(PartitionSpec("core"), PartitionSpec("core")),
            out_specs=(PartitionSpec("core"),),
            check_rep=False,
        ),
        keep_unused=True,
    )
    _CACHE["runner"] = sharded
    return sharded


def _run_fallback(big, small):
    from concourse.bass_utils import run_bass_kernel_spmd
    in_maps = [{"big": big[c], "small": small[c]} for c in range(8)]
    res = run_bass_kernel_spmd(_get_nc(), in_maps, core_ids=list(range(8)))
    return res.results[0]["o"]


def run_cores(bufs):
    big, small = bufs
    if "runner_failed" in _CACHE:
        return _run_fallback(big, small)
    try:
        sharded = _get_runner()
        out, = sharded(big.reshape(-1), small.reshape(-1))
        # every core holds the full gathered output; fetch core 0's shard only
        return np.asarray(out.addressable_data(0))
    except Exception as e:
        import sys
        print(f"kernel: jit runner failed ({type(e).__name__}: {e}); "
              f"using spmd fallback", file=sys.stderr)
        _CACHE["runner_failed"] = True
        return _run_fallback(big, small)


def assemble(out):
    # out: [8, C, NQ] bf16 (all cores' partial outputs)
    o8 = np.asarray(out).astype(np.float32).reshape(4, 2, C, ROWS_HALF, W)
    return o8.transpose(0, 2, 1, 3, 4).reshape(4, C, H, W).copy()


def kernel(**inputs):
    bufs = _prep(inputs)
    out = run_cores(bufs)
    return assemble(out)


# revision 19
# speedup vs baseline: 6.1572x; 1.4345x over previous
"""Trainium2 Bass kernel for nn_CrossAttention (B=4, C=256, H=W=48, heads=4).

Sharding: 8 cores = 4 batches x 2 halves. Queries split by row-half per
core; raw k/v split by row-half and exchanged on-device via a pair
AllGather; the folded 1x1-conv weights are sharded 1/8 per core and
8-way AllGathered. All per-core host data is packed into one bf16
tensor plus one small f32 tensor so each call stages two parameters.
The positional depthwise 3x3 conv runs on the vector engine as nine
shifted multiply-accumulates. The bf16 output is 8-way AllGathered on
device so the host fetches a single core's (full) output shard.
"""

import numpy as np
import ml_dtypes

import concourse.bass as bass
import concourse.mybir as mybir
import concourse.tile as tile
from concourse import bacc

F32 = mybir.dt.float32
BF16 = mybir.dt.bfloat16
F8 = mybir.dt.float8e4

C = 256
H = W = 48
NK = H * W            # 2304 keys
KC = NK // 128        # 18 key chunks
HEADS = 4
HD = 64
ROWS_HALF = 24        # rows per core
NQ = ROWS_HALF * W    # 1152 query positions per core
QS = 384              # query slice (8 rows)
NQS = NQ // QS        # 3 slices
QROWS = ROWS_HALF + 2  # 26 rows incl halo
NQH = QROWS * W       # 1248
NKH = NK // 2         # 1152 keys per core before exchange
EPS = 1e-5

# big (bf16) input layout, element offsets
OQ = 0
NQX = C * NQH                 # 319488
OW = OQ + NQX
NW = 4 * C * C // 8           # 32768 (1/8 of the four weight matrices)
OH = OW + NW
NH = NQH
OT = OH + NH
NT = 2 * 128
XLEN = OT + NT                # 353760

NKV = 2 * C * NKH             # 589824 fp8 elements (raw k/v own half)

# small (f32) input layout
SK = 0                        # tk  [2,128]
SV = SK + 256                 # tv  [264]  (4 heads x 66, bias in vf layout)
SP = SV + 264                 # tp  [2,128]
SJ = SP + 256                 # tj  [2,128]
SW9 = SJ + 256                # w9  [2,9,128]
SLEN = SW9 + 2304             # 3336

_CACHE = {}


def _build():
    nc = bacc.Bacc("TRN2", target_bir_lowering=False, num_devices=8)
    big = nc.dram_tensor("big", [XLEN], BF16, kind="ExternalInput")
    small = nc.dram_tensor("small", [SLEN], F32, kind="ExternalInput")
    big8 = nc.dram_tensor("big8", [NKV], F8, kind="ExternalInput")
    o = nc.dram_tensor("o", [8, C, NQ], BF16, kind="ExternalOutput")

    with tile.TileContext(nc) as tc:
        with (
            tc.tile_pool(name="dram", bufs=1, space="DRAM") as dram,
            tc.tile_pool(name="wp", bufs=1) as wp,
            tc.tile_pool(name="inp", bufs=1) as inp,
            tc.tile_pool(name="feat", bufs=1) as feat,
            tc.tile_pool(name="vfp", bufs=18) as vfp,
            tc.tile_pool(name="et", bufs=40) as etp,
            tc.tile_pool(name="small", bufs=3) as smp,
            tc.tile_pool(name="ps_s", bufs=2, space="PSUM") as ps_s,
            tc.tile_pool(name="ps_w", bufs=2, space="PSUM") as ps_w,
        ):
            # ---- weight AllGather: 1/8 slice per core -> full four matrices ----
            win = dram.tile([1, NW], BF16, tag="win")
            nc.gpsimd.dma_start(
                out=win[:], in_=big[OW:OW + NW].rearrange("(x n) -> x n", x=1))
            wall = dram.tile([4, 2, 128, C], BF16, tag="wall", addr_space="Shared")
            nc.gpsimd.collective_compute(
                "AllGather", mybir.AluOpType.bypass,
                replica_groups=[[0, 1, 2, 3, 4, 5, 6, 7]],
                ins=[win.opt()], outs=[wall.opt()],
            )
            # ---- k/v pair AllGather: own key half -> both halves (fp8) ----
            kvin = dram.tile([2, 2, 128, NKH], F8, tag="kvin")
            nc.gpsimd.dma_start(
                out=kvin[:],
                in_=big8[:].rearrange("(t a p n) -> t a p n", t=2, a=2, p=128))
            kvout = dram.tile([2, 2, 2, 128, NKH], F8, tag="kvout")
            nc.gpsimd.collective_compute(
                "AllGather", mybir.AluOpType.bypass,
                replica_groups=[[0, 1], [2, 3], [4, 5], [6, 7]],
                ins=[kvin.opt()], outs=[kvout.opt()],
            )

            # ---- inputs / weights to SBUF ----
            q_sb = inp.tile([128, 2, NQH], BF16, tag="q")
            nc.sync.dma_start(
                out=q_sb[:], in_=big[OQ:OQ + NQX].rearrange("(a p n) -> p a n", p=128, n=NQH))
            hq_sb = wp.tile([1, NQH], BF16, tag="hq")
            nc.sync.dma_start(
                out=hq_sb[:], in_=big[OH:OH + NH].rearrange("(x n) -> x n", x=1))
            tq_sb = wp.tile([1, 2, 128], BF16, tag="tq")
            nc.sync.dma_start(
                out=tq_sb[:], in_=big[OT:OT + NT].rearrange("(x a n) -> x a n", x=1, a=2))

            w_q = wp.tile([128, 2, C], BF16, tag="wq")
            w_k = wp.tile([128, 2, C], BF16, tag="wk")
            w_v = wp.tile([128, 2, C], BF16, tag="wv")
            w_p = wp.tile([128, 2, C], BF16, tag="wpj")
            for wi, t in enumerate((w_q, w_k, w_v, w_p)):
                nc.sync.dma_start(out=t[:], in_=wall[wi].rearrange("a p n -> p a n"))

            k8 = inp.tile([128, 2, NK], F8, tag="k8")
            v8 = inp.tile([128, 2, NK], F8, tag="v8")
            for hh in range(2):
                nc.sync.dma_start(
                    out=k8[:, :, hh * NKH:(hh + 1) * NKH],
                    in_=kvout[hh, 0].rearrange("a p n -> p a n"))
                nc.sync.dma_start(
                    out=v8[:, :, hh * NKH:(hh + 1) * NKH],
                    in_=kvout[hh, 1].rearrange("a p n -> p a n"))
            k_sb = inp.tile([128, 2, NK], BF16, tag="k")
            v_sb = inp.tile([128, 2, NK], BF16, tag="v")
            nc.vector.tensor_copy(k_sb[:], k8[:])
            nc.vector.tensor_copy(v_sb[:], v8[:])

            tk_sb = wp.tile([128, 2, 1], F32, tag="tk")
            nc.sync.dma_start(
                out=tk_sb[:], in_=small[SK:SK + 256].rearrange("(a p x) -> p a x", p=128, x=1))
            tp_sb = wp.tile([128, 2, 1], F32, tag="tp")
            nc.sync.dma_start(
                out=tp_sb[:], in_=small[SP:SP + 256].rearrange("(a p x) -> p a x", p=128, x=1))
            tj_sb = wp.tile([128, 2, 1], F32, tag="tj")
            nc.sync.dma_start(
                out=tj_sb[:], in_=small[SJ:SJ + 256].rearrange("(a p x) -> p a x", p=128, x=1))
            w9_sb = wp.tile([128, 2, 9], F32, tag="w9")
            nc.sync.dma_start(
                out=w9_sb[:], in_=small[SW9:SW9 + 2304].rearrange("(a t p) -> p a t", a=2, t=9))
            tv1 = wp.tile([1, 264], F32, tag="tv1")
            nc.sync.dma_start(
                out=tv1[:], in_=small[SV:SV + 264].rearrange("(x n) -> x n", x=1))
            tv_sb = wp.tile([128, 264], F32, tag="tv")
            nc.gpsimd.partition_broadcast(tv_sb[:], tv1[:])

            # ---- qf: channel-major query features (scaled), with halo rows ----
            qf = feat.tile([128, 2, NQH], BF16, tag="qf")
            for co in range(2):
                for n0 in range(0, NQH, 512):
                    nn = min(512, NQH - n0)
                    ps = ps_w.tile([128, 512], F32, tag="w")
                    for ci in range(2):
                        nc.tensor.matmul(
                            ps[:, 0:nn],
                            w_q[:, ci, co * 128:(co + 1) * 128],
                            q_sb[:, ci, n0:n0 + nn],
                            start=(ci == 0), stop=False,
                        )
                    # masked bias: qf += tq[c] * hmask[n]  (rank-1)
                    nc.tensor.matmul(
                        ps[:, 0:nn],
                        tq_sb[:, co, :],
                        hq_sb[:, n0:n0 + nn],
                        start=False, stop=True,
                    )
                    nc.vector.tensor_copy(qf[:, co, n0:n0 + nn], ps[:, 0:nn])

            # ---- kf: channel-major key features [128, 2, NK] bf16 ----
            kf = feat.tile([128, 2, NK], BF16, tag="kf")
            for co in range(2):
                for n0 in range(0, NK, 512):
                    nn = min(512, NK - n0)
                    ps = ps_w.tile([128, 512], F32, tag="w")
                    for ci in range(2):
                        nc.tensor.matmul(
                            ps[:, 0:nn],
                            w_k[:, ci, co * 128:(co + 1) * 128],
                            k_sb[:, ci, n0:n0 + nn],
                            start=(ci == 0), stop=(ci == 1),
                        )
                    nc.vector.tensor_scalar(
                        kf[:, co, n0:n0 + nn], ps[:, 0:nn],
                        tk_sb[:, co, :], None, mybir.AluOpType.add,
                    )

            # ---- vf: position-major value features, 18 tiles [128, 4, 66] ----
            # per head h: cols [v(64) | 1 | pad]
            vf = []
            for pc in range(KC):
                vt = vfp.tile([128, 4, 66], BF16, tag="vf")
                nc.vector.memset(vt[:], 1.0)
                ps = ps_w.tile([128, 512], F32, tag="w")
                for ci in range(2):
                    nc.tensor.matmul(
                        ps[:, 0:C],
                        v_sb[:, ci, pc * 128:(pc + 1) * 128],
                        w_v[:, ci, :],
                        start=(ci == 0), stop=(ci == 1),
                    )
                psv = ps[:, 0:C].rearrange("p (h d) -> p h d", h=4)
                tvv = tv_sb[:].rearrange("p (h f) -> p h f", h=4)
                nc.vector.tensor_add(vt[:, :, 0:64], psv[:], tvv[:, :, 0:64])
                vf.append(vt)

            qfr = qf[:].rearrange("p a (r w) -> p a r w", w=W)

            # ---- attention + pe + proj, software-pipelined across q slices:
            # while ACT runs exp for slice si, PE runs AV/pe/proj of si-1.
            def emit_s_group(st, t, h):
                hp, par = h // 2, h % 2
                rs = slice(par * 64, par * 64 + 64)
                s = ps_s.tile([128, 3, 512], F32, tag="s")
                for i in range(3):
                    kc = t * 3 + i
                    nc.tensor.matmul(
                        s[:, i, 0:QS],
                        kf[rs, hp, kc * 128:(kc + 1) * 128],
                        qf[rs, hp, st["q0"]:st["q0"] + QS],
                        start=True, stop=True,
                    )
                et = etp.tile([128, 3, QS], BF16, tag="et")
                nc.scalar.activation(et[:], s[:, :, 0:QS],
                                     mybir.ActivationFunctionType.Exp)
                st["ets"][t][h] = et

            def emit_av_head(st, h):
                y = ps_w.tile([128, 512], F32, tag="w")
                for t in range(6):
                    for i in range(3):
                        kc = t * 3 + i
                        nc.tensor.matmul(
                            y[0:65, 0:QS], vf[kc][:, h, 0:65],
                            st["ets"][t][h][:, i, :],
                            start=(kc == 0), stop=(kc == KC - 1),
                        )
                st["ys"][h] = y

            def emit_norm(st, pair):
                ys = [st["ys"][pair * 2], st["ys"][pair * 2 + 1]]
                ynt = smp.tile([128, QS], BF16, tag="yn")
                rr = smp.tile([1, 2, QS], F32, tag="rr")
                rq = smp.tile([128, 2, QS], F32, tag="rq")
                for par in range(2):
                    nc.vector.reciprocal(rr[:, par, :], ys[par][64:65, 0:QS])
                nc.gpsimd.partition_broadcast(rq[:], rr[:])
                nc.vector.tensor_mul(ynt[0:64, :], ys[0][0:64, 0:QS], rq[0:64, 0, :])
                nc.vector.tensor_mul(ynt[64:128, :], ys[1][0:64, 0:QS], rq[64:128, 1, :])
                st["yn"][pair] = ynt

            # pe taps ordered so the first writes the full width (dj == 0)
            PE_TAPS = [(-1, 0), (-1, -1), (-1, 1), (0, -1), (0, 0), (0, 1),
                       (1, -1), (1, 0), (1, 1)]

            def emit_tail(st):
                r0, si = st["r0"], st["si"]
                yt = [None, None]
                for ch in range(2):
                    pet = smp.tile([128, QS], F32, tag="pe")
                    pev = pet[:].rearrange("p (r w) -> p r w", w=W)
                    for idx, (di, dj) in enumerate(PE_TAPS):
                        ti = (di + 1) * 3 + (dj + 1)
                        j0o, j0i = max(0, -dj), max(0, dj)
                        ncol = W - abs(dj)
                        src = qfr[:, ch, r0 + 1 + di:r0 + 9 + di, j0i:j0i + ncol]
                        if idx == 0:
                            nc.vector.tensor_scalar(
                                pev[:, :, j0o:j0o + ncol], src,
                                w9_sb[:, ch, ti:ti + 1], None, mybir.AluOpType.mult,
                            )
                        else:
                            nc.vector.scalar_tensor_tensor(
                                out=pev[:, :, j0o:j0o + ncol], in0=src,
                                scalar=w9_sb[:, ch, ti:ti + 1],
                                in1=pev[:, :, j0o:j0o + ncol],
                                op0=mybir.AluOpType.mult, op1=mybir.AluOpType.add,
                            )
                    ytt = smp.tile([128, QS], BF16, tag="yt")
                    nc.vector.scalar_tensor_tensor(
                        out=ytt[:], in0=pet[:], scalar=tp_sb[:, ch, :],
                        in1=st["yn"][ch][:], op0=mybir.AluOpType.add,
                        op1=mybir.AluOpType.add,
                    )
                    yt[ch] = ytt
                ob = smp.tile([128, 2, QS], BF16, tag="ob")
                for co in range(2):
                    pj = ps_w.tile([128, 512], F32, tag="w")
                    for ci in range(2):
                        nc.tensor.matmul(
                            pj[:, 0:QS],
                            w_p[:, ci, co * 128:(co + 1) * 128],
                            yt[ci][:],
                            start=(ci == 0), stop=(ci == 1),
                        )
                    nc.vector.tensor_scalar(
                        ob[:, co, :], pj[:, 0:QS], tj_sb[:, co, :], None,
                        mybir.AluOpType.add,
                    )
                nc.sync.dma_start(
                    out=opart[:].rearrange("(a p) n -> p a n", p=128)[:, :, si * QS:(si + 1) * QS],
                    in_=ob[:],
                )

            opart = dram.tile([C, NQ], BF16, tag="opart")

            FIRE = {4: lambda st: emit_av_head(st, 0),
                    8: lambda st: emit_av_head(st, 1),
                    12: lambda st: emit_norm(st, 0),
                    16: lambda st: emit_av_head(st, 2),
                    20: lambda st: emit_av_head(st, 3),
                    24: lambda st: emit_norm(st, 1)}

            prev = None
            for si in range(NQS + 1):
                cur = None
                if si < NQS:
                    cur = {"si": si, "q0": 48 + si * QS, "r0": si * (QS // W),
                           "ets": [[None] * HEADS for _ in range(6)],
                           "ys": [None] * 4, "yn": [None, None]}
                    g = 0
                    for t in range(6):
                        for h in range(HEADS):
                            emit_s_group(cur, t, h)
                            g += 1
                            if prev is not None and g in FIRE:
                                FIRE[g](prev)
                    if prev is not None:
                        emit_tail(prev)
                else:
                    for g in (4, 8, 12, 16, 20, 24):
                        FIRE[g](prev)
                    emit_tail(prev)
                prev = cur

            # ---- gather all cores' outputs so the host fetches one shard ----
            ofull = dram.tile([8, C, NQ], BF16, tag="ofull", addr_space="Shared")
            nc.gpsimd.collective_compute(
                "AllGather", mybir.AluOpType.bypass,
                replica_groups=[[0, 1, 2, 3, 4, 5, 6, 7]],
                ins=[opart.opt()], outs=[ofull.opt()],
            )
            nc.gpsimd.dma_start(out=o[:], in_=ofull[:])
    nc.compile()
    return nc


def _prep(inputs):
    """Host-side: fold BN into weights, pack per-core staged buffers."""
    f64 = np.float64
    bf = ml_dtypes.bfloat16

    def fold(w, g, b, m, v):
        s = g.astype(f64) / np.sqrt(v.astype(f64) + EPS)
        return w.astype(f64) * s[:, None], b.astype(f64) - m.astype(f64) * s

    wq, tq = fold(inputs["wq_w"], inputs["wq_g"], inputs["wq_b"], inputs["wq_m"], inputs["wq_v"])
    wk, tk = fold(inputs["wk_w"], inputs["wk_g"], inputs["wk_b"], inputs["wk_m"], inputs["wk_v"])
    wv, tv = fold(inputs["wv_w"], inputs["wv_g"], inputs["wv_b"], inputs["wv_m"], inputs["wv_v"])
    wp, tj = fold(inputs["proj_w"], inputs["proj_g"], inputs["proj_b"], inputs["proj_m"], inputs["proj_v"])
    scale = 1.0 / np.sqrt(HD)
    wq, tq = wq * scale, tq * scale
    s_pe = inputs["pe_g"].astype(f64) / np.sqrt(inputs["pe_v"].astype(f64) + EPS)
    tp = inputs["pe_b"].astype(f64) - inputs["pe_m"].astype(f64) * s_pe
    w9 = inputs["pe_w"].astype(f64).reshape(C, 9) * s_pe[:, None] / scale  # pe sees unscaled qf

    # four weight matrices, transposed, flat in [4, 2, 128, C] order
    w4 = np.empty((4, C, C), dtype=bf)
    for i, m in enumerate((wq, wk, wv, wp)):
        w4[i] = m.T.astype(bf)
    w4f = w4.reshape(4 * C * C)

    # small f32 buffer (identical on every core)
    small = np.zeros(SLEN, dtype=np.float32)
    small[SK:SK + 256] = tk.astype(np.float32)
    tvv = tv.astype(np.float32).reshape(4, 64)
    svv = small[SV:SV + 264].reshape(4, 66)
    svv[:, 0:64] = tvv
    small[SP:SP + 256] = tp.astype(np.float32)
    small[SJ:SJ + 256] = tj.astype(np.float32)
    # w9 packed (a, tap, p)
    small[SW9:SW9 + 2304] = (
        w9.reshape(2, 128, 9).transpose(0, 2, 1).astype(np.float32).reshape(-1))

    f8 = ml_dtypes.float8_e4m3
    if "big" not in _CACHE:
        _CACHE["big"] = np.empty((8, XLEN), dtype=bf)
        _CACHE["small"] = np.empty((8, SLEN), dtype=np.float32)
        _CACHE["big8"] = np.empty((8, NKV), dtype=f8)
    bigb = _CACHE["big"]
    smallb = _CACHE["small"]
    big8b = _CACHE["big8"]
    smallb[:] = small[None, :]

    q = inputs["q"].astype(bf).reshape(4, C, H, W)
    k = inputs["k"].astype(f8).reshape(4, C, H, W)
    v = inputs["v"].astype(f8).reshape(4, C, H, W)
    tqb = tq.astype(bf)

    for c in range(8):
        b, half = c // 2, c % 2
        r0 = half * ROWS_HALF
        qx = bigb[c, OQ:OQ + NQX].reshape(C, QROWS, W)
        hm = np.zeros((QROWS,), dtype=bf)
        lo, hi = max(0, r0 - 1), min(H, r0 + ROWS_HALF + 1)
        a0 = lo - (r0 - 1)
        if a0 > 0:
            qx[:, 0:a0] = 0
        if a0 + (hi - lo) < QROWS:
            qx[:, a0 + (hi - lo):] = 0
        qx[:, a0:a0 + (hi - lo)] = q[b, :, lo:hi]
        hm[a0:a0 + (hi - lo)] = 1
        kv = big8b[c].reshape(2, C, NKH)
        kv[0] = k[b, :, r0:r0 + ROWS_HALF].reshape(C, NKH)
        kv[1] = v[b, :, r0:r0 + ROWS_HALF].reshape(C, NKH)
        bigb[c, OW:OW + NW] = w4f[c * NW:(c + 1) * NW]
        bigb[c, OH:OH + NH] = np.repeat(hm, W)
        bigb[c, OT:OT + NT] = tqb
    return bigb, smallb, big8b


def _get_nc():
    if "nc" not in _CACHE:
        _CACHE["nc"] = _build()
    return _CACHE["nc"]


def _get_runner():
    if "runner" in _CACHE:
        return _CACHE["runner"]
    import jax
    from jax.sharding import Mesh, PartitionSpec
    from jax.experimental.shard_map import shard_map
    from concourse import bass2jax

    nc = _get_nc()
    bass2jax.install_neuronx_cc_hook()
    out_aval = jax.core.ShapedArray((8, C, NQ), ml_dtypes.bfloat16)
    pid_name = nc.partition_id_tensor.name if nc.partition_id_tensor else None
    in_names = ("big", "small", "big8") + ((pid_name,) if pid_name else ())

    def _body(bigv, smallv, big8v):
        operands = [bigv, smallv, big8v]
        if pid_name is not None:
            operands.append(bass2jax.partition_id_tensor())
        outs = bass2jax._bass_exec_p.bind(
            *operands,
            out_avals=(out_aval,),
            in_names=in_names,
            out_names=("o",),
            lowering_input_output_aliases=(),
            sim_require_finite=True,
            sim_require_nnan=True,
            nc=nc,
        )
        return tuple(outs)

    devices = jax.devices()[:8]
    mesh = Mesh(np.asarray(devices), ("core",))
    sharded = jax.jit(
        shard_map(
            _body, mesh=mesh,
            in_specs=(PartitionSpec("core"),) * 3,
            out_specs=(PartitionSpec("core"),),
            check_rep=False,
        ),
        keep_unused=True,
    )
    _CACHE["runner"] = sharded
    return sharded


def _run_fallback(big, small, big8):
    from concourse.bass_utils import run_bass_kernel_spmd
    in_maps = [{"big": big[c], "small": small[c], "big8": big8[c]}
               for c in range(8)]
    res = run_bass_kernel_spmd(_get_nc(), in_maps, core_ids=list(range(8)))
    return res.results[0]["o"]


def run_cores(bufs):
    big, small, big8 = bufs
    if "runner_failed" in _CACHE:
        return _run_fallback(big, small, big8)
    try:
        sharded = _get_runner()
        out, = sharded(big.reshape(-1), small.reshape(-1), big8.reshape(-1))
        # every core holds the full gathered output; fetch core 0's shard only
        return np.asarray(out.addressable_data(0))
    except Exception as e:
        import sys
        print(f"kernel: jit runner failed ({type(e).__name__}: {e}); "
              f"using spmd fallback", file=sys.stderr)
        _CACHE["runner_failed"] = True
        return _run_fallback(big, small, big8)


def assemble(out):
    # out: [8, C, NQ] bf16 (all cores' partial outputs)
    o8 = np.asarray(out).astype(np.float32).reshape(4, 2, C, ROWS_HALF, W)
    return o8.transpose(0, 2, 1, 3, 4).reshape(4, C, H, W).copy()


def kernel(**inputs):
    bufs = _prep(inputs)
    out = run_cores(bufs)
    return assemble(out)


# revision 27
# speedup vs baseline: 6.7737x; 1.1001x over previous
"""Trainium2 Bass kernel for nn_CrossAttention (B=4, C=256, H=W=48, heads=4).

Sharding: 8 cores = 4 batches x 2 halves. Queries split by row-half per
core; raw k/v split by row-half and exchanged on-device via a pair
AllGather; the folded 1x1-conv weights are sharded 1/8 per core and
8-way AllGathered. All per-core host data is packed into one bf16
tensor plus one small f32 tensor so each call stages two parameters.
The positional depthwise 3x3 conv runs on the vector engine as nine
shifted multiply-accumulates. The bf16 output is 8-way AllGathered on
device so the host fetches a single core's (full) output shard.
"""

import numpy as np
import ml_dtypes

import concourse.bass as bass
import concourse.mybir as mybir
import concourse.tile as tile
from concourse import bacc

F32 = mybir.dt.float32
BF16 = mybir.dt.bfloat16
F8 = mybir.dt.float8e4

C = 256
H = W = 48
NK = H * W            # 2304 keys
KC = NK // 128        # 18 key chunks
HEADS = 4
HD = 64
ROWS_HALF = 24        # rows per core
NQ = ROWS_HALF * W    # 1152 query positions per core
QS = 384              # query slice (8 rows)
NQS = NQ // QS        # 3 slices
QROWS = ROWS_HALF + 2  # 26 rows incl halo
NQH = QROWS * W       # 1248
NKH = NK // 2         # 1152 keys per core before exchange
EPS = 1e-5

# big (bf16) input layout, element offsets
OW = 0
NW = 4 * C * C // 8           # 32768 (1/8 of the four weight matrices)
OH = OW + NW
NH = NQH
OT = OH + NH
NT = 2 * 128
XLEN = OT + NT                # 34272

# big8 (fp8) input layout
NKV = 2 * C * NKH             # 589824 (raw k/v own half)
O8Q = NKV
NQX = C * NQH                 # 319488 (raw q rows incl halo)
X8LEN = NKV + NQX             # 909312

# small (f32) input layout
SK = 0                        # tk  [2,128]
SV = SK + 256                 # tv  [264]  (4 heads x 66, bias in vf layout)
SP = SV + 264                 # tp  [2,128]
SJ = SP + 256                 # tj  [2,128]
SW9 = SJ + 256                # w9  [2,9,128]
SLEN = SW9 + 2304             # 3336

_CACHE = {}


def _build():
    nc = bacc.Bacc("TRN2", target_bir_lowering=False, num_devices=8)
    big = nc.dram_tensor("big", [XLEN], BF16, kind="ExternalInput")
    small = nc.dram_tensor("small", [SLEN], F32, kind="ExternalInput")
    big8 = nc.dram_tensor("big8", [X8LEN], F8, kind="ExternalInput")
    o = nc.dram_tensor("o", [8, C, NQ], BF16, kind="ExternalOutput")

    with tile.TileContext(nc) as tc:
        with (
            tc.tile_pool(name="dram", bufs=1, space="DRAM") as dram,
            tc.tile_pool(name="wp", bufs=1) as wp,
            tc.tile_pool(name="inp", bufs=1) as inp,
            tc.tile_pool(name="feat", bufs=1) as feat,
            tc.tile_pool(name="vfp", bufs=18) as vfp,
            tc.tile_pool(name="et", bufs=40) as etp,
            tc.tile_pool(name="small", bufs=3) as smp,
            tc.tile_pool(name="ps_s", bufs=2, space="PSUM") as ps_s,
            tc.tile_pool(name="ps_w", bufs=2, space="PSUM") as ps_w,
        ):
            # ---- weight AllGather: 1/8 slice per core -> full four matrices ----
            win = dram.tile([1, NW], BF16, tag="win")
            nc.gpsimd.dma_start(
                out=win[:], in_=big[OW:OW + NW].rearrange("(x n) -> x n", x=1))
            wall = dram.tile([4, 2, 128, C], BF16, tag="wall", addr_space="Shared")
            nc.gpsimd.collective_compute(
                "AllGather", mybir.AluOpType.bypass,
                replica_groups=[[0, 1, 2, 3, 4, 5, 6, 7]],
                ins=[win.opt()], outs=[wall.opt()],
            )
            # ---- k/v pair AllGather: own key half -> both halves (fp8) ----
            kvin = dram.tile([2, 2, 128, NKH], F8, tag="kvin")
            nc.gpsimd.dma_start(
                out=kvin[:],
                in_=big8[0:NKV].rearrange("(t a p n) -> t a p n", t=2, a=2, p=128))
            kvout = dram.tile([2, 2, 2, 128, NKH], F8, tag="kvout")
            nc.gpsimd.collective_compute(
                "AllGather", mybir.AluOpType.bypass,
                replica_groups=[[0, 1], [2, 3], [4, 5], [6, 7]],
                ins=[kvin.opt()], outs=[kvout.opt()],
            )

            # ---- inputs / weights to SBUF ----
            q8 = inp.tile([128, 2, NQH], F8, tag="q8")
            nc.sync.dma_start(
                out=q8[:], in_=big8[O8Q:O8Q + NQX].rearrange("(a p n) -> p a n", p=128, n=NQH))
            q_sb = inp.tile([128, 2, NQH], BF16, tag="q")
            nc.vector.tensor_copy(q_sb[:], q8[:])
            hq_sb = wp.tile([1, NQH], BF16, tag="hq")
            nc.sync.dma_start(
                out=hq_sb[:], in_=big[OH:OH + NH].rearrange("(x n) -> x n", x=1))
            tq_sb = wp.tile([1, 2, 128], BF16, tag="tq")
            nc.sync.dma_start(
                out=tq_sb[:], in_=big[OT:OT + NT].rearrange("(x a n) -> x a n", x=1, a=2))

            w_q = wp.tile([128, 2, C], BF16, tag="wq")
            w_k = wp.tile([128, 2, C], BF16, tag="wk")
            w_v = wp.tile([128, 2, C], BF16, tag="wv")
            w_p = wp.tile([128, 2, C], BF16, tag="wpj")
            for wi, t in enumerate((w_q, w_k, w_v, w_p)):
                nc.sync.dma_start(out=t[:], in_=wall[wi].rearrange("a p n -> p a n"))

            k8 = inp.tile([128, 2, NK], F8, tag="k8")
            v8 = inp.tile([128, 2, NK], F8, tag="v8")
            for hh in range(2):
                nc.sync.dma_start(
                    out=k8[:, :, hh * NKH:(hh + 1) * NKH],
                    in_=kvout[hh, 0].rearrange("a p n -> p a n"))
                nc.sync.dma_start(
                    out=v8[:, :, hh * NKH:(hh + 1) * NKH],
                    in_=kvout[hh, 1].rearrange("a p n -> p a n"))
            k_sb = inp.tile([128, 2, NK], BF16, tag="k")
            v_sb = inp.tile([128, 2, NK], BF16, tag="v")
            nc.vector.tensor_copy(k_sb[:], k8[:])
            nc.vector.tensor_copy(v_sb[:], v8[:])

            tk_sb = wp.tile([128, 2, 1], F32, tag="tk")
            nc.sync.dma_start(
                out=tk_sb[:], in_=small[SK:SK + 256].rearrange("(a p x) -> p a x", p=128, x=1))
            tp_sb = wp.tile([128, 2, 1], F32, tag="tp")
            nc.sync.dma_start(
                out=tp_sb[:], in_=small[SP:SP + 256].rearrange("(a p x) -> p a x", p=128, x=1))
            tj_sb = wp.tile([128, 2, 1], F32, tag="tj")
            nc.sync.dma_start(
                out=tj_sb[:], in_=small[SJ:SJ + 256].rearrange("(a p x) -> p a x", p=128, x=1))
            w9_sb = wp.tile([128, 2, 9], F32, tag="w9")
            nc.sync.dma_start(
                out=w9_sb[:], in_=small[SW9:SW9 + 2304].rearrange("(a t p) -> p a t", a=2, t=9))
            tv1 = wp.tile([1, 264], F32, tag="tv1")
            nc.sync.dma_start(
                out=tv1[:], in_=small[SV:SV + 264].rearrange("(x n) -> x n", x=1))
            tv_sb = wp.tile([128, 264], F32, tag="tv")
            nc.gpsimd.partition_broadcast(tv_sb[:], tv1[:])

            # ---- qf: channel-major query features (scaled), with halo rows ----
            qf = feat.tile([128, 2, NQH], BF16, tag="qf")
            for co in range(2):
                for n0 in range(0, NQH, 512):
                    nn = min(512, NQH - n0)
                    ps = ps_w.tile([128, 512], F32, tag="w")
                    for ci in range(2):
                        nc.tensor.matmul(
                            ps[:, 0:nn],
                            w_q[:, ci, co * 128:(co + 1) * 128],
                            q_sb[:, ci, n0:n0 + nn],
                            start=(ci == 0), stop=False,
                        )
                    # masked bias: qf += tq[c] * hmask[n]  (rank-1)
                    nc.tensor.matmul(
                        ps[:, 0:nn],
                        tq_sb[:, co, :],
                        hq_sb[:, n0:n0 + nn],
                        start=False, stop=True,
                    )
                    nc.vector.tensor_copy(qf[:, co, n0:n0 + nn], ps[:, 0:nn])

            # ---- kf: channel-major key features [128, 2, NK] bf16 ----
            kf = feat.tile([128, 2, NK], BF16, tag="kf")
            for co in range(2):
                for n0 in range(0, NK, 512):
                    nn = min(512, NK - n0)
                    ps = ps_w.tile([128, 512], F32, tag="w")
                    for ci in range(2):
                        nc.tensor.matmul(
                            ps[:, 0:nn],
                            w_k[:, ci, co * 128:(co + 1) * 128],
                            k_sb[:, ci, n0:n0 + nn],
                            start=(ci == 0), stop=(ci == 1),
                        )
                    nc.vector.tensor_scalar(
                        kf[:, co, n0:n0 + nn], ps[:, 0:nn],
                        tk_sb[:, co, :], None, mybir.AluOpType.add,
                    )

            # ---- vf: position-major value features, 18 tiles [128, 4, 66] ----
            # per head h: cols [v(64) | 1 | pad]
            vf = []
            for pc in range(KC):
                vt = vfp.tile([128, 4, 66], BF16, tag="vf")
                nc.vector.memset(vt[:], 1.0)
                ps = ps_w.tile([128, 512], F32, tag="w")
                for ci in range(2):
                    nc.tensor.matmul(
                        ps[:, 0:C],
                        v_sb[:, ci, pc * 128:(pc + 1) * 128],
                        w_v[:, ci, :],
                        start=(ci == 0), stop=(ci == 1),
                    )
                psv = ps[:, 0:C].rearrange("p (h d) -> p h d", h=4)
                tvv = tv_sb[:].rearrange("p (h f) -> p h f", h=4)
                nc.vector.tensor_add(vt[:, :, 0:64], psv[:], tvv[:, :, 0:64])
                vf.append(vt)

            qfr = qf[:].rearrange("p a (r w) -> p a r w", w=W)

            # ---- attention + pe + proj, software-pipelined across q slices:
            # while ACT runs exp for slice si, PE runs AV/pe/proj of si-1.
            def emit_s_group(st, t, h):
                hp, par = h // 2, h % 2
                rs = slice(par * 64, par * 64 + 64)
                s = ps_s.tile([128, 3, 512], F32, tag="s")
                for i in range(3):
                    kc = t * 3 + i
                    nc.tensor.matmul(
                        s[:, i, 0:QS],
                        kf[rs, hp, kc * 128:(kc + 1) * 128],
                        qf[rs, hp, st["q0"]:st["q0"] + QS],
                        start=True, stop=True,
                    )
                et = etp.tile([128, 3, QS], BF16, tag="et")
                nc.scalar.activation(et[:], s[:, :, 0:QS],
                                     mybir.ActivationFunctionType.Exp)
                st["ets"][t][h] = et

            def emit_av_head(st, h):
                y = ps_w.tile([128, 512], F32, tag="w")
                for t in range(6):
                    for i in range(3):
                        kc = t * 3 + i
                        nc.tensor.matmul(
                            y[0:65, 0:QS], vf[kc][:, h, 0:65],
                            st["ets"][t][h][:, i, :],
                            start=(kc == 0), stop=(kc == KC - 1),
                        )
                st["ys"][h] = y

            def emit_norm(st, pair):
                ys = [st["ys"][pair * 2], st["ys"][pair * 2 + 1]]
                ynt = smp.tile([128, QS], BF16, tag="yn")
                rr = smp.tile([1, 2, QS], F32, tag="rr")
                rq = smp.tile([128, 2, QS], F32, tag="rq")
                for par in range(2):
                    nc.vector.reciprocal(rr[:, par, :], ys[par][64:65, 0:QS])
                nc.gpsimd.partition_broadcast(rq[:], rr[:])
                nc.vector.tensor_mul(ynt[0:64, :], ys[0][0:64, 0:QS], rq[0:64, 0, :])
                nc.vector.tensor_mul(ynt[64:128, :], ys[1][0:64, 0:QS], rq[64:128, 1, :])
                st["yn"][pair] = ynt

            # pe taps ordered so the first writes the full width (dj == 0)
            PE_TAPS = [(-1, 0), (-1, -1), (-1, 1), (0, -1), (0, 0), (0, 1),
                       (1, -1), (1, 0), (1, 1)]

            def emit_tail(st):
                r0, si = st["r0"], st["si"]
                yt = [None, None]
                for ch in range(2):
                    pet = smp.tile([128, QS], F32, tag="pe")
                    pev = pet[:].rearrange("p (r w) -> p r w", w=W)
                    for idx, (di, dj) in enumerate(PE_TAPS):
                        ti = (di + 1) * 3 + (dj + 1)
                        j0o, j0i = max(0, -dj), max(0, dj)
                        ncol = W - abs(dj)
                        src = qfr[:, ch, r0 + 1 + di:r0 + 9 + di, j0i:j0i + ncol]
                        if idx == 0:
                            nc.vector.tensor_scalar(
                                pev[:, :, j0o:j0o + ncol], src,
                                w9_sb[:, ch, ti:ti + 1], None, mybir.AluOpType.mult,
                            )
                        else:
                            nc.vector.scalar_tensor_tensor(
                                out=pev[:, :, j0o:j0o + ncol], in0=src,
                                scalar=w9_sb[:, ch, ti:ti + 1],
                                in1=pev[:, :, j0o:j0o + ncol],
                                op0=mybir.AluOpType.mult, op1=mybir.AluOpType.add,
                            )
                    ytt = smp.tile([128, QS], BF16, tag="yt")
                    nc.vector.scalar_tensor_tensor(
                        out=ytt[:], in0=pet[:], scalar=tp_sb[:, ch, :],
                        in1=st["yn"][ch][:], op0=mybir.AluOpType.add,
                        op1=mybir.AluOpType.add,
                    )
                    yt[ch] = ytt
                ob = smp.tile([128, 2, QS], BF16, tag="ob")
                for co in range(2):
                    pj = ps_w.tile([128, 512], F32, tag="w")
                    for ci in range(2):
                        nc.tensor.matmul(
                            pj[:, 0:QS],
                            w_p[:, ci, co * 128:(co + 1) * 128],
                            yt[ci][:],
                            start=(ci == 0), stop=(ci == 1),
                        )
                    nc.vector.tensor_scalar(
                        ob[:, co, :], pj[:, 0:QS], tj_sb[:, co, :], None,
                        mybir.AluOpType.add,
                    )
                nc.sync.dma_start(
                    out=opart[:].rearrange("(a p) n -> p a n", p=128)[:, :, si * QS:(si + 1) * QS],
                    in_=ob[:],
                )

            opart = dram.tile([C, NQ], BF16, tag="opart")

            FIRE = {4: lambda st: emit_av_head(st, 0),
                    8: lambda st: emit_av_head(st, 1),
                    12: lambda st: emit_norm(st, 0),
                    16: lambda st: emit_av_head(st, 2),
                    20: lambda st: emit_av_head(st, 3),
                    24: lambda st: emit_norm(st, 1)}

            prev = None
            for si in range(NQS + 1):
                cur = None
                if si < NQS:
                    cur = {"si": si, "q0": 48 + si * QS, "r0": si * (QS // W),
                           "ets": [[None] * HEADS for _ in range(6)],
                           "ys": [None] * 4, "yn": [None, None]}
                    g = 0
                    for t in range(6):
                        for h in range(HEADS):
                            emit_s_group(cur, t, h)
                            g += 1
                            if prev is not None and g in FIRE:
                                FIRE[g](prev)
                    if prev is not None:
                        emit_tail(prev)
                else:
                    for g in (4, 8, 12, 16, 20, 24):
                        FIRE[g](prev)
                    emit_tail(prev)
                prev = cur

            # ---- gather all cores' outputs so the host fetches one shard ----
            ofull = dram.tile([8, C, NQ], BF16, tag="ofull", addr_space="Shared")
            nc.gpsimd.collective_compute(
                "AllGather", mybir.AluOpType.bypass,
                replica_groups=[[0, 1, 2, 3, 4, 5, 6, 7]],
                ins=[opart.opt()], outs=[ofull.opt()],
            )
            nc.gpsimd.dma_start(out=o[:], in_=ofull[:])
    nc.compile()
    return nc


def _prep(inputs):
    """Host-side: fold BN into weights, pack per-core staged buffers."""
    f64 = np.float64
    bf = ml_dtypes.bfloat16

    def fold(w, g, b, m, v):
        s = g.astype(f64) / np.sqrt(v.astype(f64) + EPS)
        return w.astype(f64) * s[:, None], b.astype(f64) - m.astype(f64) * s

    wq, tq = fold(inputs["wq_w"], inputs["wq_g"], inputs["wq_b"], inputs["wq_m"], inputs["wq_v"])
    wk, tk = fold(inputs["wk_w"], inputs["wk_g"], inputs["wk_b"], inputs["wk_m"], inputs["wk_v"])
    wv, tv = fold(inputs["wv_w"], inputs["wv_g"], inputs["wv_b"], inputs["wv_m"], inputs["wv_v"])
    wp, tj = fold(inputs["proj_w"], inputs["proj_g"], inputs["proj_b"], inputs["proj_m"], inputs["proj_v"])
    scale = 1.0 / np.sqrt(HD)
    wq, tq = wq * scale, tq * scale
    s_pe = inputs["pe_g"].astype(f64) / np.sqrt(inputs["pe_v"].astype(f64) + EPS)
    tp = inputs["pe_b"].astype(f64) - inputs["pe_m"].astype(f64) * s_pe
    w9 = inputs["pe_w"].astype(f64).reshape(C, 9) * s_pe[:, None] / scale  # pe sees unscaled qf

    # four weight matrices, transposed, flat in [4, 2, 128, C] order
    w4 = np.empty((4, C, C), dtype=bf)
    for i, m in enumerate((wq, wk, wv, wp)):
        w4[i] = m.T.astype(bf)
    w4f = w4.reshape(4 * C * C)

    # small f32 buffer (identical on every core)
    small = np.zeros(SLEN, dtype=np.float32)
    small[SK:SK + 256] = tk.astype(np.float32)
    tvv = tv.astype(np.float32).reshape(4, 64)
    svv = small[SV:SV + 264].reshape(4, 66)
    svv[:, 0:64] = tvv
    small[SP:SP + 256] = tp.astype(np.float32)
    small[SJ:SJ + 256] = tj.astype(np.float32)
    # w9 packed (a, tap, p)
    small[SW9:SW9 + 2304] = (
        w9.reshape(2, 128, 9).transpose(0, 2, 1).astype(np.float32).reshape(-1))

    f8 = ml_dtypes.float8_e4m3
    if "big" not in _CACHE:
        _CACHE["big"] = np.empty((8, XLEN), dtype=bf)
        _CACHE["small"] = np.empty((8, SLEN), dtype=np.float32)
        _CACHE["big8"] = np.empty((8, X8LEN), dtype=f8)
    bigb = _CACHE["big"]
    smallb = _CACHE["small"]
    big8b = _CACHE["big8"]
    smallb[:] = small[None, :]

    q = inputs["q"].astype(f8).reshape(4, C, H, W)
    k = inputs["k"].astype(f8).reshape(4, C, H, W)
    v = inputs["v"].astype(f8).reshape(4, C, H, W)
    tqb = tq.astype(bf)

    for c in range(8):
        b, half = c // 2, c % 2
        r0 = half * ROWS_HALF
        qx = big8b[c, O8Q:O8Q + NQX].reshape(C, QROWS, W)
        hm = np.zeros((QROWS,), dtype=bf)
        lo, hi = max(0, r0 - 1), min(H, r0 + ROWS_HALF + 1)
        a0 = lo - (r0 - 1)
        if a0 > 0:
            qx[:, 0:a0] = 0
        if a0 + (hi - lo) < QROWS:
            qx[:, a0 + (hi - lo):] = 0
        qx[:, a0:a0 + (hi - lo)] = q[b, :, lo:hi]
        hm[a0:a0 + (hi - lo)] = 1
        kv = big8b[c, 0:NKV].reshape(2, C, NKH)
        kv[0] = k[b, :, r0:r0 + ROWS_HALF].reshape(C, NKH)
        kv[1] = v[b, :, r0:r0 + ROWS_HALF].reshape(C, NKH)
        bigb[c, OW:OW + NW] = w4f[c * NW:(c + 1) * NW]
        bigb[c, OH:OH + NH] = np.repeat(hm, W)
        bigb[c, OT:OT + NT] = tqb
    return bigb, smallb, big8b


def _get_nc():
    if "nc" not in _CACHE:
        _CACHE["nc"] = _build()
    return _CACHE["nc"]


def _get_runner():
    if "runner" in _CACHE:
        return _CACHE["runner"]
    import jax
    from jax.sharding import Mesh, PartitionSpec
    from jax.experimental.shard_map import shard_map
    from concourse import bass2jax

    nc = _get_nc()
    bass2jax.install_neuronx_cc_hook()
    out_aval = jax.core.ShapedArray((8, C, NQ), ml_dtypes.bfloat16)
    pid_name = nc.partition_id_tensor.name if nc.partition_id_tensor else None
    in_names = ("big", "small", "big8") + ((pid_name,) if pid_name else ())

    def _body(bigv, smallv, big8v):
        operands = [bigv, smallv, big8v]
        if pid_name is not None:
            operands.append(bass2jax.partition_id_tensor())
        outs = bass2jax._bass_exec_p.bind(
            *operands,
            out_avals=(out_aval,),
            in_names=in_names,
            out_names=("o",),
            lowering_input_output_aliases=(),
            sim_require_finite=True,
            sim_require_nnan=True,
            nc=nc,
        )
        return tuple(outs)

    devices = jax.devices()[:8]
    mesh = Mesh(np.asarray(devices), ("core",))
    sharded = jax.jit(
        shard_map(
            _body, mesh=mesh,
            in_specs=(PartitionSpec("core"),) * 3,
            out_specs=(PartitionSpec("core"),),
            check_rep=False,
        ),
        keep_unused=True,
    )
    _CACHE["runner"] = sharded
    return sharded


def _run_fallback(big, small, big8):
    from concourse.bass_utils import run_bass_kernel_spmd
    in_maps = [{"big": big[c], "small": small[c], "big8": big8[c]}
               for c in range(8)]
    res = run_bass_kernel_spmd(_get_nc(), in_maps, core_ids=list(range(8)))
    return res.results[0]["o"]


def run_cores(bufs):
    big, small, big8 = bufs
    if "runner_failed" in _CACHE:
        return _run_fallback(big, small, big8)
    try:
        sharded = _get_runner()
        out, = sharded(big.reshape(-1), small.reshape(-1), big8.reshape(-1))
        # every core holds the full gathered output; fetch core 0's shard only
        return np.asarray(out.addressable_data(0))
    except Exception as e:
        import sys
        print(f"kernel: jit runner failed ({type(e).__name__}: {e}); "
              f"using spmd fallback", file=sys.stderr)
        _CACHE["runner_failed"] = True
        return _run_fallback(big, small, big8)


def assemble(out):
    # out: [8, C, NQ] bf16 (all cores' partial outputs)
    o8 = np.asarray(out).astype(np.float32).reshape(4, 2, C, ROWS_HALF, W)
    return o8.transpose(0, 2, 1, 3, 4).reshape(4, C, H, W).copy()


def kernel(**inputs):
    bufs = _prep(inputs)
    out = run_cores(bufs)
    return assemble(out)
